# revision 14
# baseline (speedup 1.0000x reference)
"""Trainium2 Bass kernel for nn_Classifier_1821066133734 (GDAS NAS cell).

Strategy
--------
* The gumbel-softmax routing is global (per-edge, not per-token): compute it
  on host in numpy, exactly mirroring the fp32 reference ops, and specialize
  the device program to the selected op per edge.  hardwts[e, index[e]] is
  (1 - p) + p in fp32 (== 1.0 for all realistic inputs); a generic scale path
  exists for the w != 1 case.
* Data-parallel over 8 NeuronCores: batch 16384 -> 2048 rows per core.
  All tensors live TRANSPOSED on device: channels on SBUF partitions, batch in
  the free dimension.  Host pre-transposes x and the selected weights, so the
  device program needs no transposes at all.
* Matmuls run as float32r (TF32-class, 1 cycle/row at N=512 -- measured 227ns
  per [128k x 128m x 512n] LDW+MM pair warm, vs 852ns for fp32).
* BatchNorm normalizes over the FULL batch.  BN only occurs inside pool ops.
  BN(BN(t)) collapses analytically: mean2 = 0, var2 = v1/(v1+eps), so every
  pool edge is a per-channel affine of a "source" tensor (t = relu(x) @ pwT
  for A-edges, the source state for B-edges) with coefficients derived from
  global per-channel sum/sumsq.  Those stats need one tiny AllReduce
  (n_pool_sources x 512 x 2 floats) across the 8 cores.
* Two-segment schedule: segment 1 computes nodes 1..Q (Q = max pool source),
  the pool source tensors and their local stats, spilling live states to DRAM;
  then the stats AllReduce; segment 2 computes nodes Q+1..6 including the
  pool affines, streaming the spilled states back per chunk.
"""

import os

import numpy as np

NODE_NUM = 6
C_IN = 1024
CH = 512
B = 16384
N_CORES = 8
BL = B // N_CORES  # 2048
N_OPS = 9
TAU = np.float32(10.0)
EPS = 1e-5
P = 128
CT = CH // P  # 4 channel partition-tiles
XT = C_IN // P  # 8

# Results of the traced run (filled when KERNEL_TRACE=1), for test.py.
LAST_RESULTS = None

_PROGRAM_CACHE = {}


# ---------------------------------------------------------------------------
# Host-side routing (numpy mirror of the jax reference)
# ---------------------------------------------------------------------------

def _routing(arch_params, gumbel):
    ap = np.asarray(arch_params, dtype=np.float32)
    gm = np.asarray(gumbel, dtype=np.float32)
    m = ap.max(axis=1, keepdims=True)
    s = ap - m
    lse = np.log(np.exp(s).sum(axis=1, keepdims=True), dtype=np.float32)
    logp = s - lse
    logits = (logp + gm) / TAU
    mm = logits.max(axis=1, keepdims=True)
    e = np.exp(logits - mm, dtype=np.float32)
    prob = e / e.sum(axis=1, keepdims=True)
    index = prob.argmax(axis=1)
    p = prob[np.arange(prob.shape[0]), index].astype(np.float32)
    w = (np.float32(1.0) - p) + p  # forward value of the straight-through wt
    return index, w


def _edge_list(index, w_sel):
    """Edges in reference iteration order with their selected op."""
    keys = sorted(
        "{}->{}".format(j, i + 1) for i in range(NODE_NUM) for j in range(i + 1)
    )
    e2i = {k: n for n, k in enumerate(keys)}
    edges = []
    a_ct = 0
    b_ct = 0
    for i in range(1, NODE_NUM + 1):
        for j in range(i):
            row = e2i["{}->{}".format(j, i)]
            kind = "A" if j == 0 else "B"
            rec = {
                "row": row,
                "src": j,
                "dst": i,
                "kind": kind,
                "slot": a_ct if kind == "A" else b_ct,
                "op": int(index[row]),
                "w": float(w_sel[row]),
            }
            if kind == "A":
                a_ct += 1
            else:
                b_ct += 1
            edges.append(rec)
    return edges


# ---------------------------------------------------------------------------
# Device program builder
# ---------------------------------------------------------------------------

def _plan(edges):
    """Segment plan: list of (nodes, barrier_after: bool)."""
    pool_edges = [e for e in edges if e["op"] in (0, 1)]
    if not pool_edges:
        return [list(range(1, NODE_NUM + 1))], []
    avail = [0 if e["kind"] == "A" else e["src"] for e in pool_edges]
    dests = [e["dst"] for e in pool_edges]
    q = max(avail)
    if q < min(dests):
        barriers = [q]
    else:
        # fallback: a barrier right before every pool dest (correct, slower)
        barriers = sorted({d - 1 for d in dests})
    segs = []
    prev = 0
    for bp in barriers:
        segs.append(list(range(prev + 1, bp + 1)))
        prev = bp
    segs.append(list(range(prev + 1, NODE_NUM + 1)))
    segs = [s for s in segs if s]
    return segs, barriers


def _build_program(cfg):
    """cfg: tuple of (op, w_is_one) per edge in reference order."""
    import concourse.mybir as mybir
    import concourse.tile as tile
    from concourse import bacc
    from concourse.bass import ts

    R = mybir.dt.float32r
    F = mybir.dt.float32
    AF = mybir.ActivationFunctionType
    ALU = mybir.AluOpType
    AX = mybir.AxisListType

    index = [c[0] for c in cfg]
    w_one = [c[1] for c in cfg]
    edges = _edge_list(index, [1.0] * len(index))
    # edges hold no numeric w; the actual scale value comes in through a small
    # "scales" input tensor so w != 1 doesn't force a recompile.
    for e in edges:
        e["w_one"] = bool(w_one[e["row"]])

    stage = int(os.environ.get("KERNEL_STAGE", "9"))
    feats = set(os.environ.get("KERNEL_FEAT", "").split(","))
    if stage <= 6:
        edges = [e for e in edges if e["dst"] <= stage]
        if stage <= 3:
            edges = [e for e in edges if e["op"] not in (0, 1)]
    segs, barriers = _plan(edges)
    if stage <= 6:
        segs = [[n for n in seg if n <= stage] for seg in segs]
        segs = [seg for seg in segs if seg]
    pool_edges = [e for e in edges if e["op"] in (0, 1)]
    # pool sources: ("t", edge_idx) for A pools, ("s", src_node) for B pools
    src_keys = []
    for e in pool_edges:
        key = ("t", e["row"]) if e["kind"] == "A" else ("s", e["src"])
        e["src_key"] = key
        if key not in src_keys:
            src_keys.append(key)
    n_src = len(src_keys)

    # which states must be spilled: used in a later segment than produced
    node_seg = {}
    for si, seg in enumerate(segs):
        for n in seg:
            node_seg[n] = si
    spill_states = set()
    if len(segs) > 1:
        for e in edges:
            if e["op"] == 8:
                continue
            if e["src"] >= 1 and node_seg[e["dst"]] > node_seg[e["src"]]:
                spill_states.add(e["src"])
        for key in src_keys:
            if key[0] == "s" and node_seg[key[1]] < len(segs) - 1:
                spill_states.add(key[1])

    nc = bacc.Bacc("TRN2", target_bir_lowering=False, debug=False,
                   num_devices=N_CORES)

    # ---- DRAM parameters -------------------------------------------------
    x_d = nc.declare_dram_parameter("x", [XT, P, BL], R, isOutput=False)
    w_d = {}
    n_bias = 0
    bias_col = {}
    for e in edges:
        op = e["op"]
        ei = e["row"]
        if op in (2, 3, 4):  # dense
            kt = XT if e["kind"] == "A" else CT
            w_d[ei] = nc.declare_dram_parameter(f"w{ei}", [kt, P, CH], R,
                                                isOutput=False)
            bias_col[ei] = n_bias
            n_bias += 1
        elif op in (5, 6, 7):  # grouped dense
            kt = 8 if e["kind"] == "A" else 4  # 4 groups x (2 or 1) ktiles
            w_d[ei] = nc.declare_dram_parameter(f"w{ei}", [kt, P, P], R,
                                                isOutput=False)
            bias_col[ei] = n_bias
            n_bias += 1
        elif op in (0, 1) and e["kind"] == "A":  # pool preprocess matmul
            w_d[ei] = nc.declare_dram_parameter(f"w{ei}", [XT, P, CH], R,
                                                isOutput=False)
    n_bias = max(n_bias, 1)
    bias_d = nc.declare_dram_parameter("biases", [CT, P, n_bias], F,
                                       isOutput=False)
    # per-edge scale (hardwts value); only read on the w != 1 path
    scale_d = nc.declare_dram_parameter("scales", [1, len(edges)], F,
                                        isOutput=False)
    out_d = nc.declare_dram_parameter("out", [P, CT, BL], F, isOutput=True)

    seg_of_edge = {}
    for e in edges:
        seg_of_edge[e["row"]] = node_seg[e["dst"]]

    with tile.TileContext(nc) as tc:
        with (
            tc.tile_pool(name="persist", bufs=1) as pp,
            tc.tile_pool(name="dram", bufs=1, space="DRAM") as dp,
        ):
            bias_sb = pp.tile([P, CT, n_bias], F)
            for ct in range(CT):
                nc.sync.dma_start(bias_sb[:, ct, :], bias_d[ct])
            scale_sb = None
            if not all(e["w_one"] for e in edges):
                scale_sb = pp.tile([P, len(edges)], F)
                nc.sync.dma_start(scale_sb[:1, :], scale_d[:])

            # spill DRAM tensors
            spill_d = {}
            for n in sorted(spill_states):
                spill_d[n] = dp.tile([P, CT, BL], R, name=f"spill_s{n}")
            t_spill_d = {}
            for key in src_keys:
                if key[0] == "t":
                    t_spill_d[key[1]] = dp.tile([P, CT, BL], F,
                                                name=f"spill_t{key[1]}")

            # stats columns: per source, per ct, [sum over chunks..., sq...]
            max_chunks = 4
            stat_sum = pp.tile([P, max(n_src, 1), CT, max_chunks], F)
            stat_sq = pp.tile([P, max(n_src, 1), CT, max_chunks], F)
            # collective buffers
            cc_in = dp.tile([P, max(n_src, 1) * CT * 2], F, name="cc_in")
            cc_out = dp.tile([P, max(n_src, 1) * CT * 2], F,
                             name="cc_out", addr_space="Shared")
            # BN affine coefficients: per pool edge (scale, const)
            coef = pp.tile([P, max(len(pool_edges), 1), CT, 2], F)
            # per-dest total const
            pool_dests = sorted({e["dst"] for e in pool_edges})
            ctot = pp.tile([P, max(len(pool_dests), 1), CT], F)

            states = {}  # node -> sbuf tile for current chunk (seg-local)

            def dense_mm(psum_t, w_sb, src_sb, kts, nb, nh, cp):
                # kt outer, batch-halves inner: both MMs of a half-pair share
                # one weight load (LDWEIGHTS amortization)
                for i, kt in enumerate(kts):
                    for h in range(nh):
                        nc.tensor.matmul(
                            psum_t[:, h * 512:(h + 1) * 512],
                            w_sb[:, kt, ts(cp, P)],
                            src_sb[:, kt, h * 512:h * 512 + 512],
                            start=(i == 0),
                            stop=(i == len(kts) - 1),
                        )

            def epilogue(e, psum_t, dest, ct, first, nb, act_idx,
                         tmp_pool, accum_out=None):
                """dest[:, ct, :nb] (+)= act(psum + bias) [* w]"""
                ei = e["row"]
                bias_ap = bias_sb[:, ct, bias_col[ei]:bias_col[ei] + 1]
                w1 = e["w_one"]
                dst_ap = dest[:, ct, 0:nb]
                if act_idx == 0:  # relu on DVE
                    if first and w1:
                        nc.vector.tensor_scalar(
                            dst_ap, psum_t, bias_ap, 0.0, ALU.add, ALU.max)
                    else:
                        tmp = tmp_pool.tile([P, nb], R, tag="tmp")
                        nc.vector.tensor_scalar(
                            tmp[:], psum_t, bias_ap, 0.0, ALU.add, ALU.max)
                        _accum(e, dst_ap, tmp[:], first)
                else:
                    func = AF.Sigmoid if act_idx == 1 else AF.Tanh
                    if first and w1:
                        nc.scalar.activation(dst_ap, psum_t, func,
                                             bias=bias_ap, scale=1.0,
                                             accum_out=accum_out)
                    else:
                        tmp = tmp_pool.tile([P, nb], R, tag="tmp")
                        nc.scalar.activation(tmp[:], psum_t, func,
                                             bias=bias_ap, scale=1.0)
                        _accum(e, dst_ap, tmp[:], first)

            def _accum(e, dst_ap, tmp_ap, first):
                if first:
                    if e["w_one"]:
                        nc.vector.tensor_copy(dst_ap, tmp_ap)
                    else:
                        nc.vector.tensor_scalar_mul(
                            dst_ap, tmp_ap,
                            scale_sb[:1, e["row"]:e["row"] + 1]
                            .partition_broadcast(P))
                else:
                    if e["w_one"]:
                        nc.vector.tensor_add(dst_ap, dst_ap, tmp_ap)
                    else:
                        nc.vector.scalar_tensor_tensor(
                            dst_ap, tmp_ap,
                            scale_sb[:1, e["row"]:e["row"] + 1]
                            .partition_broadcast(P),
                            dst_ap, ALU.mult, ALU.add)

            # ---------------- segments ----------------
            barrier_done = 0
            for si, seg in enumerate(segs):
                is_last_seg = si == len(segs) - 1
                nb = min(1024, BL)
                nchunks = BL // nb
                nh = nb // 512

                seg_edges = [e for e in edges
                             if node_seg[e["dst"]] == si and e["op"] != 8]
                # pool-source t edges whose stats barrier is at end of this seg
                t_edges = [e for e in pool_edges
                           if e["kind"] == "A" and si == 0]
                need_x = any(e["src"] == 0 and e["op"] != 8 for e in seg_edges) \
                    or bool(t_edges)

                with (
                    tc.tile_pool(name=f"w{si}", bufs=1) as wp,
                    tc.tile_pool(name=f"st{si}", bufs=1) as sp,
                    tc.tile_pool(name=f"tmp{si}", bufs=3) as tp,
                    tc.tile_pool(name=f"ps{si}", bufs=3,
                                 space="PSUM") as psp,
                ):
                    # -- weights: preload when the segment set is small,
                    #    stream per chunk through a shared 2-slot pool when big
                    seg_w_edges = []
                    seen_w = set()
                    for e in [x for x in seg_edges
                              if x["op"] not in (0, 1)] + t_edges:
                        ei = e["row"]
                        if ei in w_d and ei not in seen_w:
                            seen_w.add(ei)
                            seg_w_edges.append(ei)
                    w_kb = sum(w_d[ei].shape[0] * w_d[ei].shape[2] * 4 // 1024
                               for ei in seg_w_edges)
                    stream_w = w_kb > 60
                    w_sb = {}

                    def load_w(ei):
                        shp = w_d[ei].shape
                        t_w = wp.tile([P, shp[0], shp[2]], R,
                                      name=f"wsb{ei}",
                                      tag=("wstream" if stream_w
                                           else f"wsb{ei}"),
                                      bufs=(2 if stream_w else 1))
                        for kt in range(shp[0]):
                            nc.sync.dma_start(t_w[:, kt, :], w_d[ei][kt])
                        return t_w

                    if not stream_w:
                        for ei in seg_w_edges:
                            w_sb[ei] = load_w(ei)

                    for c in range(nchunks):
                        if stream_w:
                            w_sb = {}
                        bsl = slice(c * nb, (c + 1) * nb)
                        # -- load x chunk --
                        if need_x:
                            x_sb = sp.tile([P, XT, nb], R, tag="x")
                            for kt in range(XT):
                                nc.sync.dma_start(x_sb[:, kt, :],
                                                  x_d[kt, :, bsl])
                        # -- reload spilled states used this segment --
                        for n in sorted(spill_states):
                            if node_seg[n] < si and any(
                                    e["src"] == n for e in seg_edges):
                                st = sp.tile([P, CT, nb], R, tag=f"s{n}")
                                nc.sync.dma_start(st[:], spill_d[n][:, :, bsl])
                                states[n] = st
                        # -- reload t tensors for affines this segment --
                        t_tiles = {}
                        for e in pool_edges:
                            if "nopoolaff" in feats:
                                break
                            if e["kind"] == "A" and node_seg[e["dst"]] == si \
                                    and si > 0:
                                tt = sp.tile([P, CT, nb], F, tag=f"t{e['row']}")
                                nc.sync.dma_start(
                                    tt[:], t_spill_d[e["row"]][:, :, bsl])
                                t_tiles[e["row"]] = tt

                        # -- compute nodes --
                        for node in seg:
                            n_edges = [e for e in edges if e["dst"] == node
                                       and e["op"] != 8]
                            reg = [e for e in n_edges if e["op"] not in (0, 1)]
                            pools = [e for e in n_edges if e["op"] in (0, 1)]
                            last_node = segs[-1][-1]
                            if node < last_node:
                                acc = sp.tile([P, CT, nb], R, tag=f"s{node}")
                            else:
                                acc = sp.tile([P, CT, nb], F, tag="acc_f")
                            states[node] = acc
                            first = [True] * CT
                            for e in reg:
                                op = e["op"]
                                src_sb = x_sb if e["src"] == 0 \
                                    else states[e["src"]]
                                act_idx = op - 2 if op in (2, 3, 4) else op - 5
                                if op in (2, 3, 4):  # dense
                                    kts = list(range(
                                        XT if e["kind"] == "A" else CT))
                                    if e["row"] not in w_sb:
                                        w_sb[e["row"]] = load_w(e["row"])
                                    for cp in range(CT):
                                        ps = psp.tile([P, nb], F, tag="ps")
                                        dense_mm(ps, w_sb[e["row"]], src_sb,
                                                 kts, nb, nh, cp)
                                        epilogue(e, ps[:], acc, cp,
                                                 first[cp], nb, act_idx, tp)
                                        first[cp] = False
                                else:  # grouped dense
                                    ktg = 2 if e["kind"] == "A" else 1
                                    if e["row"] not in w_sb:
                                        w_sb[e["row"]] = load_w(e["row"])
                                    for g in range(4):
                                        ps = psp.tile([P, nb], F, tag="ps")
                                        for i in range(ktg):
                                            for h in range(nh):
                                                h0 = h * 512
                                                nc.tensor.matmul(
                                                    ps[:, h0:h0 + 512],
                                                    w_sb[e["row"]][:, g * ktg + i, :],
                                                    src_sb[:, g * ktg + i,
                                                           h0:h0 + 512],
                                                    start=(i == 0),
                                                    stop=(i == ktg - 1),
                                                )
                                        epilogue(e, ps[:], acc, g,
                                                 first[g], nb, act_idx, tp)
                                        first[g] = False
                            # pool affines (coefficients ready after barrier)
                            if "nopoolaff" in feats:
                                pools = []
                            for pi, e in enumerate(pools):
                                pe_i = pool_edges.index(e)
                                if e["kind"] == "A":
                                    src_t = t_tiles[e["row"]]
                                else:
                                    src_t = states[e["src"]]
                                last_pool = pi == len(pools) - 1
                                di = pool_dests.index(node)
                                for ct in range(CT):
                                    sc = coef[:, pe_i, ct, 0:1]
                                    if not last_pool:
                                        assert not first[ct]
                                        nc.vector.scalar_tensor_tensor(
                                            acc[:, ct, :], src_t[:, ct, :],
                                            sc, acc[:, ct, :],
                                            ALU.mult, ALU.add)
                                    else:
                                        v = tp.tile([P, nb], F, tag="tmp")
                                        nc.vector.tensor_scalar(
                                            v[:], src_t[:, ct, :], sc,
                                            ctot[:, di, ct:ct + 1],
                                            ALU.mult, ALU.add)
                                        if first[ct]:
                                            nc.vector.tensor_copy(
                                                acc[:, ct, :], v[:])
                                        else:
                                            nc.vector.tensor_add(
                                                acc[:, ct, :], acc[:, ct, :],
                                                v[:])
                                if last_pool:
                                    for ct in range(CT):
                                        first[ct] = False
                            if node == last_node:
                                nc.sync.dma_start(out_d[:, :, bsl], acc[:])

                        # -- pool source t tensors + stats (first segment) --
                        if "nostats" in feats:
                            t_edges = []
                        if si == 0 and t_edges:
                            # relu(x) in place (x is dead after this)
                            if "noreluip" in feats:
                                rx = sp.tile([P, XT, nb], R, tag="rx")
                                for kt in range(XT):
                                    nc.vector.tensor_scalar_max(
                                        rx[:, kt, :], x_sb[:, kt, :], 0.0)
                                x_sb = rx
                            else:
                                for kt in range(XT):
                                    nc.vector.tensor_scalar_max(
                                        x_sb[:, kt, :], x_sb[:, kt, :], 0.0)
                        for e in t_edges:
                            tt = sp.tile([P, CT, nb], F, tag=f"t{e['row']}")
                            ski = src_keys.index(e["src_key"])
                            if e["row"] not in w_sb:
                                w_sb[e["row"]] = load_w(e["row"])
                            for cp in range(CT):
                                ps = psp.tile([P, nb], F, tag="ps")
                                dense_mm(ps, w_sb[e["row"]], x_sb,
                                         list(range(XT)), nb, nh, cp)
                                nc.scalar.activation(
                                    tt[:, cp, :], ps[:], AF.Copy,
                                    bias=0.0, scale=1.0,
                                    accum_out=(None if "noacc" in feats else
                                               stat_sum[:, ski, cp, c:c + 1]))
                                if "nottr" not in feats:
                                    sq = tp.tile([P, nb], F, tag="tmp")
                                    nc.vector.scalar_tensor_tensor(
                                        sq[:], tt[:, cp, :], 1.0, tt[:, cp, :],
                                        ALU.mult, ALU.mult,
                                        accum_out=stat_sq[:, ski, cp, c:c + 1])
                            nc.sync.dma_start(
                                t_spill_d[e["row"]][:, :, bsl], tt[:])
                        # -- stats of B-pool source states in this segment --
                        if si < len(segs) - 1 and "nostats" not in feats:
                            for ski, key in enumerate(src_keys):
                                if key[0] != "s" or node_seg[key[1]] != si:
                                    continue
                                st = states[key[1]]
                                for cp in range(CT):
                                    nc.vector.reduce_sum(
                                        stat_sum[:, ski, cp, c:c + 1],
                                        st[:, cp, :].bitcast(F), axis=AX.X)
                                    if "nottr" not in feats:
                                        sq = tp.tile([P, nb], F, tag="tmp")
                                        nc.vector.scalar_tensor_tensor(
                                            sq[:], st[:, cp, :].bitcast(F), 1.0,
                                            st[:, cp, :].bitcast(F),
                                            ALU.mult, ALU.mult,
                                            accum_out=stat_sq[:, ski, cp, c:c + 1])
                        # -- spill states produced here and needed later --
                        for n in seg:
                            if n in spill_states:
                                nc.sync.dma_start(
                                    spill_d[n][:, :, bsl], states[n][:])

                    # ---- barrier: allreduce stats, compute coefficients ----
                    if si < len(segs) - 1 and n_src > 0 and barrier_done == 0 \
                            and "nostats" not in feats \
                            and "nobarrier" not in feats:
                        barrier_done = 1
                        packed = pp.tile([P, n_src, CT, 2], F)
                        for ski in range(n_src):
                            for cp in range(CT):
                                nc.vector.reduce_sum(
                                    packed[:, ski, cp, 0:1],
                                    stat_sum[:, ski, cp, 0:nchunks], axis=AX.X)
                                nc.vector.reduce_sum(
                                    packed[:, ski, cp, 1:2],
                                    stat_sq[:, ski, cp, 0:nchunks], axis=AX.X)
                        no_cc = os.environ.get("KERNEL_NO_CC", "0") == "1"
                        nc.sync.dma_start(
                            cc_in[:, 0:n_src * CT * 2],
                            packed[:].rearrange("p a b c -> p (a b c)"))
                        if no_cc:
                            nc.sync.dma_start(cc_out[:, 0:n_src * CT * 2],
                                              cc_in[:, 0:n_src * CT * 2])
                        else:
                            nc.gpsimd.collective_compute(
                                "AllReduce", mybir.AluOpType.add,
                                ins=[cc_in.opt()], outs=[cc_out.opt()],
                                replica_groups=[list(range(N_CORES))],
                            )
                        red = pp.tile([P, n_src, CT, 2], F)
                        nc.sync.dma_start(
                            red[:].rearrange("p a b c -> p (a b c)"),
                            cc_out[:, 0:n_src * CT * 2])
                        # coefficient computation (tiny [P, CT] tensors)
                        invB = (1.0 / B) if not no_cc else (1.0 / BL)
                        sc1 = pp.tile([P, n_src, CT, 8], F)  # scratch
                        for ski, key in enumerate(src_keys):
                            mS = sc1[:, ski, :, 0]    # mean
                            vS = sc1[:, ski, :, 1]    # var
                            uS = sc1[:, ski, :, 2]    # var+eps
                            ruS = sc1[:, ski, :, 3]   # 1/(var+eps)
                            r1S = sc1[:, ski, :, 4]   # rsqrt(var+eps)
                            t5 = sc1[:, ski, :, 5]
                            t6 = sc1[:, ski, :, 6]
                            t7 = sc1[:, ski, :, 7]
                            sm = red[:, ski, :, 0]
                            sq_ = red[:, ski, :, 1]
                            nc.vector.tensor_scalar_mul(mS, sm, invB)
                            nc.vector.tensor_scalar_mul(vS, sq_, invB)
                            nc.vector.tensor_mul(t5, mS, mS)
                            nc.vector.tensor_sub(vS, vS, t5)
                            nc.vector.tensor_scalar_add(uS, vS, EPS)
                            nc.vector.reciprocal(ruS, uS)
                            nc.scalar.activation(r1S, ruS, AF.Sqrt)
                        for pe_i, e in enumerate(pool_edges):
                            ski = src_keys.index(e["src_key"])
                            mS = sc1[:, ski, :, 0]
                            vS = sc1[:, ski, :, 1]
                            ruS = sc1[:, ski, :, 3]
                            r1S = sc1[:, ski, :, 4]
                            t5 = sc1[:, ski, :, 5]
                            t6 = sc1[:, ski, :, 6]
                            t7 = sc1[:, ski, :, 7]
                            scl = coef[:, pe_i, :, 0]
                            cst = coef[:, pe_i, :, 1]
                            if e["kind"] == "A":
                                # v2 = v/(v+eps); avg: v2 /= 81
                                nc.vector.tensor_mul(t5, vS, ruS)
                                if e["op"] == 0:
                                    nc.vector.tensor_scalar_mul(
                                        t5, t5, 1.0 / 81.0)
                                nc.vector.tensor_scalar_add(t5, t5, EPS)
                                nc.vector.reciprocal(t6, t5)
                                nc.scalar.activation(t7, t6, AF.Sqrt)
                                nc.vector.tensor_mul(scl, r1S, t7)
                                if e["op"] == 0:
                                    nc.vector.tensor_scalar_mul(
                                        scl, scl, 1.0 / 9.0)
                            else:
                                if e["op"] == 0:
                                    nc.vector.tensor_scalar_mul(
                                        t5, vS, 1.0 / 81.0)
                                    nc.vector.tensor_scalar_add(t5, t5, EPS)
                                    nc.vector.reciprocal(t6, t5)
                                    nc.scalar.activation(t7, t6, AF.Sqrt)
                                    nc.vector.tensor_scalar_mul(
                                        scl, t7, 1.0 / 9.0)
                                else:
                                    nc.vector.tensor_copy(scl, r1S)
                            if not e["w_one"]:
                                nc.vector.tensor_scalar_mul(
                                    scl, scl,
                                    scale_sb[:1, e["row"]:e["row"] + 1]
                                    .partition_broadcast(P))
                            nc.vector.tensor_mul(cst, mS, scl)
                            nc.vector.tensor_scalar_mul(cst, cst, -1.0)
                        for di, dnode in enumerate(pool_dests):
                            dps = [pe_i for pe_i, e in enumerate(pool_edges)
                                   if e["dst"] == dnode]
                            nc.vector.tensor_copy(ctot[:, di, :],
                                                  coef[:, dps[0], :, 1])
                            for pe_i in dps[1:]:
                                nc.vector.tensor_add(ctot[:, di, :],
                                                     ctot[:, di, :],
                                                     coef[:, pe_i, :, 1])

    nc.compile()
    return nc


# ---------------------------------------------------------------------------
# Host-side weight packing
# ---------------------------------------------------------------------------

def _pack_inputs(edges, inputs):
    arrs = {}
    bias_list = []
    for e in edges:
        op = e["op"]
        ei = e["row"]
        k = e["kind"]
        slot = e["slot"]
        if op in (2, 3, 4):
            a = op - 2
            W = np.asarray(inputs["dense_w_A" if k == "A" else "dense_w_B"]
                           )[slot, a]
            bias = np.asarray(inputs["dense_b_A" if k == "A" else "dense_b_B"]
                              )[slot, a]
            kt = XT if k == "A" else CT
            arrs[f"w{ei}"] = np.ascontiguousarray(
                W.T.reshape(kt, P, CH).astype(np.float32))
            bias_list.append((ei, bias))
        elif op in (5, 6, 7):
            a = op - 5
            gw = np.asarray(inputs["group_w_A" if k == "A" else "group_w_B"]
                            )[slot, a]  # [4, 128, cin_g]
            gb = np.asarray(inputs["group_b_A" if k == "A" else "group_b_B"]
                            )[slot, a]  # [512]
            ktg = 2 if k == "A" else 1
            wT = np.concatenate([gw[g].T for g in range(4)], axis=0)
            arrs[f"w{ei}"] = np.ascontiguousarray(
                wT.reshape(4 * ktg, P, P).astype(np.float32))
            bias_list.append((ei, gb))
        elif op in (0, 1) and k == "A":
            pw = np.asarray(inputs["pool_w_A"])[slot, op]  # [512, 1024]
            arrs[f"w{ei}"] = np.ascontiguousarray(
                pw.T.reshape(XT, P, CH).astype(np.float32))
    n_bias = max(len(bias_list), 1)
    biases = np.zeros((CT, P, n_bias), np.float32)
    col = 0
    for ei, b in bias_list:
        biases[:, :, col] = np.asarray(b, np.float32).reshape(CT, P)
        col += 1
    arrs["biases"] = biases
    return arrs


def kernel(**inputs):
    global LAST_RESULTS
    from concourse.bass_utils import run_bass_kernel_spmd

    index, w_sel = _routing(inputs["arch_params"], inputs["gumbel"])
    edges = _edge_list(index, w_sel)
    cfg = tuple((int(index[i]), bool(w_sel[i] == 1.0))
                for i in range(len(index)))

    if cfg not in _PROGRAM_CACHE:
        _PROGRAM_CACHE[cfg] = _build_program(cfg)
    nc = _PROGRAM_CACHE[cfg]

    arrs = _pack_inputs(edges, inputs)
    arrs["scales"] = np.asarray(w_sel, np.float32).reshape(1, -1)

    x = np.asarray(inputs["x"], np.float32)  # [B, C_IN]
    x_cores = x.reshape(N_CORES, BL, C_IN)

    in_maps = []
    for c in range(N_CORES):
        m = dict(arrs)
        m["x"] = np.ascontiguousarray(
            x_cores[c].T.reshape(XT, P, BL))
        in_maps.append(m)

    trace = os.environ.get("KERNEL_TRACE", "0") == "1"
    res = None
    for attempt in range(4):
        try:
            res = run_bass_kernel_spmd(nc, in_maps, list(range(N_CORES)),
                                       trace=trace)
            break
        except Exception:
            # the axon tunnel to the device pool is occasionally flaky
            # (transient "worker hung up" / INTERNAL); retry a few times
            if attempt == 3:
                raise
            import time as _time

            _time.sleep(5.0)
    LAST_RESULTS = res

    out = np.empty((B, CH), np.float32)
    for c in range(N_CORES):
        oc = res.results[c]["out"]  # [P, CT, BL]
        out[c * BL:(c + 1) * BL] = (
            oc.transpose(2, 1, 0).reshape(BL, CH))
    return out


# revision 15
# speedup vs baseline: 1.0852x; 1.0852x over previous
"""Trainium2 Bass kernel for nn_Classifier_1821066133734 (GDAS NAS cell).

Strategy
--------
* The gumbel-softmax routing is global (per-edge, not per-token): compute it
  on host in numpy, exactly mirroring the fp32 reference ops, and specialize
  the device program to the selected op per edge.  hardwts[e, index[e]] is
  (1 - p) + p in fp32 (== 1.0 for all realistic inputs); a generic scale path
  exists for the w != 1 case.
* Data-parallel over 8 NeuronCores: batch 16384 -> 2048 rows per core.
  All tensors live TRANSPOSED on device: channels on SBUF partitions, batch in
  the free dimension.  Host pre-transposes x and the selected weights, so the
  device program needs no transposes at all.
* Matmuls run as float32r (TF32-class, 1 cycle/row at N=512 -- measured 227ns
  per [128k x 128m x 512n] LDW+MM pair warm, vs 852ns for fp32).
* BatchNorm normalizes over the FULL batch.  BN only occurs inside pool ops.
  BN(BN(t)) collapses analytically: mean2 = 0, var2 = v1/(v1+eps), so every
  pool edge is a per-channel affine of a "source" tensor (t = relu(x) @ pwT
  for A-edges, the source state for B-edges) with coefficients derived from
  global per-channel sum/sumsq.  Those stats need one tiny AllReduce
  (n_pool_sources x 512 x 2 floats) across the 8 cores.
* Two-segment schedule: segment 1 computes nodes 1..Q (Q = max pool source),
  the pool source tensors and their local stats, spilling live states to DRAM;
  then the stats AllReduce; segment 2 computes nodes Q+1..6 including the
  pool affines, streaming the spilled states back per chunk.
"""

import os

import numpy as np

NODE_NUM = 6
C_IN = 1024
CH = 512
B = 16384
N_CORES = 8
BL = B // N_CORES  # 2048
N_OPS = 9
TAU = np.float32(10.0)
EPS = 1e-5
P = 128
CT = CH // P  # 4 channel partition-tiles
XT = C_IN // P  # 8

# Results of the traced run (filled when KERNEL_TRACE=1), for test.py.
LAST_RESULTS = None

_PROGRAM_CACHE = {}


# ---------------------------------------------------------------------------
# Host-side routing (numpy mirror of the jax reference)
# ---------------------------------------------------------------------------

def _routing(arch_params, gumbel):
    ap = np.asarray(arch_params, dtype=np.float32)
    gm = np.asarray(gumbel, dtype=np.float32)
    m = ap.max(axis=1, keepdims=True)
    s = ap - m
    lse = np.log(np.exp(s).sum(axis=1, keepdims=True), dtype=np.float32)
    logp = s - lse
    logits = (logp + gm) / TAU
    mm = logits.max(axis=1, keepdims=True)
    e = np.exp(logits - mm, dtype=np.float32)
    prob = e / e.sum(axis=1, keepdims=True)
    index = prob.argmax(axis=1)
    p = prob[np.arange(prob.shape[0]), index].astype(np.float32)
    w = (np.float32(1.0) - p) + p  # forward value of the straight-through wt
    return index, w


def _edge_list(index, w_sel):
    """Edges in reference iteration order with their selected op."""
    keys = sorted(
        "{}->{}".format(j, i + 1) for i in range(NODE_NUM) for j in range(i + 1)
    )
    e2i = {k: n for n, k in enumerate(keys)}
    edges = []
    a_ct = 0
    b_ct = 0
    for i in range(1, NODE_NUM + 1):
        for j in range(i):
            row = e2i["{}->{}".format(j, i)]
            kind = "A" if j == 0 else "B"
            rec = {
                "row": row,
                "src": j,
                "dst": i,
                "kind": kind,
                "slot": a_ct if kind == "A" else b_ct,
                "op": int(index[row]),
                "w": float(w_sel[row]),
            }
            if kind == "A":
                a_ct += 1
            else:
                b_ct += 1
            edges.append(rec)
    return edges


# ---------------------------------------------------------------------------
# Device program builder
# ---------------------------------------------------------------------------

def _plan(edges):
    """Segment plan: list of (nodes, barrier_after: bool)."""
    pool_edges = [e for e in edges if e["op"] in (0, 1)]
    if not pool_edges:
        return [list(range(1, NODE_NUM + 1))], []
    avail = [0 if e["kind"] == "A" else e["src"] for e in pool_edges]
    dests = [e["dst"] for e in pool_edges]
    q = max(avail)
    if q < min(dests):
        barriers = [q]
    else:
        # fallback: a barrier right before every pool dest (correct, slower)
        barriers = sorted({d - 1 for d in dests})
    segs = []
    prev = 0
    for bp in barriers:
        segs.append(list(range(prev + 1, bp + 1)))
        prev = bp
    segs.append(list(range(prev + 1, NODE_NUM + 1)))
    segs = [s for s in segs if s]
    return segs, barriers


def _build_program(cfg):
    """cfg: tuple of (op, w_is_one) per edge in reference order."""
    import concourse.mybir as mybir
    import concourse.tile as tile
    from concourse import bacc
    from concourse.bass import ts

    R = mybir.dt.float32r
    F = mybir.dt.float32
    AF = mybir.ActivationFunctionType
    ALU = mybir.AluOpType
    AX = mybir.AxisListType

    index = [c[0] for c in cfg]
    w_one = [c[1] for c in cfg]
    edges = _edge_list(index, [1.0] * len(index))
    # edges hold no numeric w; the actual scale value comes in through a small
    # "scales" input tensor so w != 1 doesn't force a recompile.
    for e in edges:
        e["w_one"] = bool(w_one[e["row"]])

    stage = int(os.environ.get("KERNEL_STAGE", "9"))
    feats = set(os.environ.get("KERNEL_FEAT", "").split(","))
    if stage <= 6:
        edges = [e for e in edges if e["dst"] <= stage]
        if stage <= 3:
            edges = [e for e in edges if e["op"] not in (0, 1)]
    segs, barriers = _plan(edges)
    if stage <= 6:
        segs = [[n for n in seg if n <= stage] for seg in segs]
        segs = [seg for seg in segs if seg]
    pool_edges = [e for e in edges if e["op"] in (0, 1)]
    # pool sources: ("t", edge_idx) for A pools, ("s", src_node) for B pools
    src_keys = []
    for e in pool_edges:
        key = ("t", e["row"]) if e["kind"] == "A" else ("s", e["src"])
        e["src_key"] = key
        if key not in src_keys:
            src_keys.append(key)
    n_src = len(src_keys)

    # which states must be spilled: used in a later segment than produced
    node_seg = {}
    for si, seg in enumerate(segs):
        for n in seg:
            node_seg[n] = si
    spill_states = set()
    if len(segs) > 1:
        for e in edges:
            if e["op"] == 8:
                continue
            if e["src"] >= 1 and node_seg[e["dst"]] > node_seg[e["src"]]:
                spill_states.add(e["src"])
        for key in src_keys:
            if key[0] == "s" and node_seg[key[1]] < len(segs) - 1:
                spill_states.add(key[1])

    nc = bacc.Bacc("TRN2", target_bir_lowering=False, debug=False,
                   num_devices=N_CORES)

    # ---- DRAM parameters -------------------------------------------------
    x_d = nc.declare_dram_parameter("x", [XT, P, BL], R, isOutput=False)
    w_d = {}
    n_bias = 0
    bias_col = {}
    for e in edges:
        op = e["op"]
        ei = e["row"]
        if op in (2, 3, 4):  # dense
            kt = XT if e["kind"] == "A" else CT
            w_d[ei] = nc.declare_dram_parameter(f"w{ei}", [kt, P, CH], R,
                                                isOutput=False)
            bias_col[ei] = n_bias
            n_bias += 1
        elif op in (5, 6, 7):  # grouped dense
            kt = 8 if e["kind"] == "A" else 4  # 4 groups x (2 or 1) ktiles
            w_d[ei] = nc.declare_dram_parameter(f"w{ei}", [kt, P, P], R,
                                                isOutput=False)
            bias_col[ei] = n_bias
            n_bias += 1
        elif op in (0, 1) and e["kind"] == "A":  # pool preprocess matmul
            w_d[ei] = nc.declare_dram_parameter(f"w{ei}", [XT, P, CH], R,
                                                isOutput=False)
    n_bias = max(n_bias, 1)
    bias_d = nc.declare_dram_parameter("biases", [CT, P, n_bias], F,
                                       isOutput=False)
    # per-edge scale (hardwts value); only read on the w != 1 path
    scale_d = nc.declare_dram_parameter("scales", [1, len(edges)], F,
                                        isOutput=False)
    out_d = nc.declare_dram_parameter("out", [P, CT, BL], F, isOutput=True)

    seg_of_edge = {}
    for e in edges:
        seg_of_edge[e["row"]] = node_seg[e["dst"]]

    with tile.TileContext(nc) as tc:
        with (
            tc.tile_pool(name="persist", bufs=1) as pp,
            tc.tile_pool(name="dram", bufs=1, space="DRAM") as dp,
        ):
            bias_sb = pp.tile([P, CT, n_bias], F)
            for ct in range(CT):
                nc.sync.dma_start(bias_sb[:, ct, :], bias_d[ct])
            scale_sb = None
            if not all(e["w_one"] for e in edges):
                scale_sb = pp.tile([P, len(edges)], F)
                nc.sync.dma_start(scale_sb[:1, :], scale_d[:])

            # spill DRAM tensors
            spill_d = {}
            for n in sorted(spill_states):
                spill_d[n] = dp.tile([P, CT, BL], R, name=f"spill_s{n}")
            t_spill_d = {}
            for key in src_keys:
                if key[0] == "t":
                    t_spill_d[key[1]] = dp.tile([P, CT, BL], F,
                                                name=f"spill_t{key[1]}")

            # stats columns: per source, per ct, [sum over chunks..., sq...]
            max_chunks = 4
            stat_sum = pp.tile([P, max(n_src, 1), CT, max_chunks], F)
            stat_sq = pp.tile([P, max(n_src, 1), CT, max_chunks], F)
            # collective buffers
            cc_in = dp.tile([P, max(n_src, 1) * CT * 2], F, name="cc_in")
            cc_out = dp.tile([P, max(n_src, 1) * CT * 2], F,
                             name="cc_out", addr_space="Shared")
            # BN affine coefficients: per pool edge (scale, const)
            coef = pp.tile([P, max(len(pool_edges), 1), CT, 2], F)
            # per-dest total const
            pool_dests = sorted({e["dst"] for e in pool_edges})
            ctot = pp.tile([P, max(len(pool_dests), 1), CT], F)

            states = {}  # node -> sbuf tile for current chunk (seg-local)

            def dense_mm(psum_t, w_sb, src_sb, kts, nb, nh, cp):
                # kt outer, batch-halves inner: both MMs of a half-pair share
                # one weight load (LDWEIGHTS amortization)
                for i, kt in enumerate(kts):
                    for h in range(nh):
                        nc.tensor.matmul(
                            psum_t[:, h * 512:(h + 1) * 512],
                            w_sb[:, kt, ts(cp, P)],
                            src_sb[:, kt, h * 512:h * 512 + 512],
                            start=(i == 0),
                            stop=(i == len(kts) - 1),
                        )

            def epilogue(e, psum_t, dest, ct, first, nb, act_idx,
                         tmp_pool, accum_out=None):
                """dest[:, ct, :nb] (+)= act(psum + bias) [* w]"""
                ei = e["row"]
                bias_ap = bias_sb[:, ct, bias_col[ei]:bias_col[ei] + 1]
                w1 = e["w_one"]
                dst_ap = dest[:, ct, 0:nb]
                if act_idx == 0:  # relu on DVE
                    if first and w1:
                        nc.vector.tensor_scalar(
                            dst_ap, psum_t, bias_ap, 0.0, ALU.add, ALU.max)
                    else:
                        tmp = tmp_pool.tile([P, nb], R, tag="tmp")
                        nc.vector.tensor_scalar(
                            tmp[:], psum_t, bias_ap, 0.0, ALU.add, ALU.max)
                        _accum(e, dst_ap, tmp[:], first)
                else:
                    func = AF.Sigmoid if act_idx == 1 else AF.Tanh
                    if first and w1:
                        nc.scalar.activation(dst_ap, psum_t, func,
                                             bias=bias_ap, scale=1.0,
                                             accum_out=accum_out)
                    else:
                        tmp = tmp_pool.tile([P, nb], R, tag="tmp")
                        nc.scalar.activation(tmp[:], psum_t, func,
                                             bias=bias_ap, scale=1.0)
                        _accum(e, dst_ap, tmp[:], first)

            def _accum(e, dst_ap, tmp_ap, first):
                if first:
                    if e["w_one"]:
                        nc.vector.tensor_copy(dst_ap, tmp_ap)
                    else:
                        nc.vector.tensor_scalar_mul(
                            dst_ap, tmp_ap,
                            scale_sb[:1, e["row"]:e["row"] + 1]
                            .partition_broadcast(P))
                else:
                    if e["w_one"]:
                        nc.vector.tensor_add(dst_ap, dst_ap, tmp_ap)
                    else:
                        nc.vector.scalar_tensor_tensor(
                            dst_ap, tmp_ap,
                            scale_sb[:1, e["row"]:e["row"] + 1]
                            .partition_broadcast(P),
                            dst_ap, ALU.mult, ALU.add)

            # ---------------- segments ----------------
            barrier_done = 0
            for si, seg in enumerate(segs):
                is_last_seg = si == len(segs) - 1
                nb = min(1024, BL)
                nchunks = BL // nb
                nh = nb // 512

                seg_edges = [e for e in edges
                             if node_seg[e["dst"]] == si and e["op"] != 8]
                # pool-source t edges whose stats barrier is at end of this seg
                t_edges = [e for e in pool_edges
                           if e["kind"] == "A" and si == 0]
                need_x = any(e["src"] == 0 and e["op"] != 8 for e in seg_edges) \
                    or bool(t_edges)

                with (
                    tc.tile_pool(name=f"w{si}", bufs=1) as wp,
                    tc.tile_pool(name=f"st{si}", bufs=1) as sp,
                    tc.tile_pool(name=f"tmp{si}", bufs=3) as tp,
                    tc.tile_pool(name=f"ps{si}", bufs=3,
                                 space="PSUM") as psp,
                ):
                    # -- weights: preload when the segment set is small,
                    #    stream per chunk through a shared 2-slot pool when big
                    seg_w_edges = []
                    seen_w = set()
                    for e in [x for x in seg_edges
                              if x["op"] not in (0, 1)] + t_edges:
                        ei = e["row"]
                        if ei in w_d and ei not in seen_w:
                            seen_w.add(ei)
                            seg_w_edges.append(ei)
                    w_kb = sum(w_d[ei].shape[0] * w_d[ei].shape[2] * 4 // 1024
                               for ei in seg_w_edges)
                    stream_w = w_kb > 60
                    w_sb = {}

                    def load_w(ei):
                        shp = w_d[ei].shape
                        t_w = wp.tile([P, shp[0], shp[2]], R,
                                      name=f"wsb{ei}",
                                      tag=("wstream" if stream_w
                                           else f"wsb{ei}"),
                                      bufs=(2 if stream_w else 1))
                        for kt in range(shp[0]):
                            nc.sync.dma_start(t_w[:, kt, :], w_d[ei][kt])
                        return t_w

                    if not stream_w:
                        for ei in seg_w_edges:
                            w_sb[ei] = load_w(ei)

                    for c in range(nchunks):
                        if stream_w:
                            w_sb = {}
                        bsl = slice(c * nb, (c + 1) * nb)
                        # lazy loads: emit each DMA at first use so the
                        # sync-queue order matches compute order (avoids the
                        # PE stalling on a FIFO of not-yet-needed transfers)
                        _x_tile = [None]

                        def get_x():
                            if _x_tile[0] is None:
                                xt_ = sp.tile([P, XT, nb], R, tag="x")
                                for kt in range(XT):
                                    nc.sync.dma_start(xt_[:, kt, :],
                                                      x_d[kt, :, bsl])
                                _x_tile[0] = xt_
                            return _x_tile[0]

                        _reloaded = set()

                        def get_state(n):
                            if node_seg[n] < si and n not in _reloaded:
                                _reloaded.add(n)
                                st = sp.tile([P, CT, nb], R, tag=f"s{n}")
                                nc.sync.dma_start(st[:],
                                                  spill_d[n][:, :, bsl])
                                states[n] = st
                            return states[n]

                        t_tiles = {}

                        def get_t(row):
                            if row not in t_tiles:
                                tt = sp.tile([P, CT, nb], F, tag=f"t{row}")
                                nc.sync.dma_start(
                                    tt[:], t_spill_d[row][:, :, bsl])
                                t_tiles[row] = tt
                            return t_tiles[row]

                        # -- compute nodes --
                        for node in seg:
                            n_edges = [e for e in edges if e["dst"] == node
                                       and e["op"] != 8]
                            reg = [e for e in n_edges if e["op"] not in (0, 1)]
                            pools = [e for e in n_edges if e["op"] in (0, 1)]
                            last_node = segs[-1][-1]
                            if node < last_node:
                                acc = sp.tile([P, CT, nb], R, tag=f"s{node}")
                            else:
                                acc = sp.tile([P, CT, nb], F, tag="acc_f")
                            states[node] = acc
                            first = [True] * CT
                            for e in reg:
                                op = e["op"]
                                src_sb = get_x() if e["src"] == 0 \
                                    else get_state(e["src"])
                                act_idx = op - 2 if op in (2, 3, 4) else op - 5
                                if op in (2, 3, 4):  # dense
                                    kts = list(range(
                                        XT if e["kind"] == "A" else CT))
                                    if e["row"] not in w_sb:
                                        w_sb[e["row"]] = load_w(e["row"])
                                    for cp in range(CT):
                                        ps = psp.tile([P, nb], F, tag="ps")
                                        dense_mm(ps, w_sb[e["row"]], src_sb,
                                                 kts, nb, nh, cp)
                                        epilogue(e, ps[:], acc, cp,
                                                 first[cp], nb, act_idx, tp)
                                        first[cp] = False
                                else:  # grouped dense
                                    ktg = 2 if e["kind"] == "A" else 1
                                    if e["row"] not in w_sb:
                                        w_sb[e["row"]] = load_w(e["row"])
                                    for g in range(4):
                                        ps = psp.tile([P, nb], F, tag="ps")
                                        for i in range(ktg):
                                            for h in range(nh):
                                                h0 = h * 512
                                                nc.tensor.matmul(
                                                    ps[:, h0:h0 + 512],
                                                    w_sb[e["row"]][:, g * ktg + i, :],
                                                    src_sb[:, g * ktg + i,
                                                           h0:h0 + 512],
                                                    start=(i == 0),
                                                    stop=(i == ktg - 1),
                                                )
                                        epilogue(e, ps[:], acc, g,
                                                 first[g], nb, act_idx, tp)
                                        first[g] = False
                            # pool affines (coefficients ready after barrier)
                            if "nopoolaff" in feats:
                                pools = []
                            for pi, e in enumerate(pools):
                                pe_i = pool_edges.index(e)
                                if e["kind"] == "A":
                                    src_t = (get_t(e["row"]) if si > 0
                                             else t_tiles[e["row"]])
                                else:
                                    src_t = get_state(e["src"])
                                last_pool = pi == len(pools) - 1
                                di = pool_dests.index(node)
                                for ct in range(CT):
                                    sc = coef[:, pe_i, ct, 0:1]
                                    if not last_pool:
                                        assert not first[ct]
                                        nc.vector.scalar_tensor_tensor(
                                            acc[:, ct, :], src_t[:, ct, :],
                                            sc, acc[:, ct, :],
                                            ALU.mult, ALU.add)
                                    else:
                                        v = tp.tile([P, nb], F, tag="tmp")
                                        nc.vector.tensor_scalar(
                                            v[:], src_t[:, ct, :], sc,
                                            ctot[:, di, ct:ct + 1],
                                            ALU.mult, ALU.add)
                                        if first[ct]:
                                            nc.vector.tensor_copy(
                                                acc[:, ct, :], v[:])
                                        else:
                                            nc.vector.tensor_add(
                                                acc[:, ct, :], acc[:, ct, :],
                                                v[:])
                                if last_pool:
                                    for ct in range(CT):
                                        first[ct] = False
                            if node == last_node:
                                nc.sync.dma_start(out_d[:, :, bsl], acc[:])

                        # -- pool source t tensors + stats (first segment) --
                        if "nostats" in feats:
                            t_edges = []
                        if si == 0 and t_edges:
                            # relu(x) in place (x is dead after this)
                            x_sb = get_x()
                            for kt in range(XT):
                                nc.vector.tensor_scalar_max(
                                    x_sb[:, kt, :], x_sb[:, kt, :], 0.0)
                        for e in t_edges:
                            tt = sp.tile([P, CT, nb], F, tag=f"t{e['row']}")
                            ski = src_keys.index(e["src_key"])
                            if e["row"] not in w_sb:
                                w_sb[e["row"]] = load_w(e["row"])
                            for cp in range(CT):
                                ps = psp.tile([P, nb], F, tag="ps")
                                dense_mm(ps, w_sb[e["row"]], x_sb,
                                         list(range(XT)), nb, nh, cp)
                                nc.scalar.activation(
                                    tt[:, cp, :], ps[:], AF.Copy,
                                    bias=0.0, scale=1.0,
                                    accum_out=(None if "noacc" in feats else
                                               stat_sum[:, ski, cp, c:c + 1]))
                                if "nottr" not in feats:
                                    sq = tp.tile([P, nb], F, tag="tmp")
                                    nc.vector.scalar_tensor_tensor(
                                        sq[:], tt[:, cp, :], 1.0, tt[:, cp, :],
                                        ALU.mult, ALU.mult,
                                        accum_out=stat_sq[:, ski, cp, c:c + 1])
                            nc.sync.dma_start(
                                t_spill_d[e["row"]][:, :, bsl], tt[:])
                        # -- stats of B-pool source states in this segment --
                        if si < len(segs) - 1 and "nostats" not in feats:
                            for ski, key in enumerate(src_keys):
                                if key[0] != "s" or node_seg[key[1]] != si:
                                    continue
                                st = states[key[1]]
                                for cp in range(CT):
                                    nc.vector.reduce_sum(
                                        stat_sum[:, ski, cp, c:c + 1],
                                        st[:, cp, :].bitcast(F), axis=AX.X)
                                    if "nottr" not in feats:
                                        sq = tp.tile([P, nb], F, tag="tmp")
                                        nc.vector.scalar_tensor_tensor(
                                            sq[:], st[:, cp, :].bitcast(F), 1.0,
                                            st[:, cp, :].bitcast(F),
                                            ALU.mult, ALU.mult,
                                            accum_out=stat_sq[:, ski, cp, c:c + 1])
                        # -- spill states produced here and needed later --
                        for n in seg:
                            if n in spill_states:
                                nc.sync.dma_start(
                                    spill_d[n][:, :, bsl], states[n][:])

                    # ---- barrier: allreduce stats, compute coefficients ----
                    if si < len(segs) - 1 and n_src > 0 and barrier_done == 0 \
                            and "nostats" not in feats \
                            and "nobarrier" not in feats:
                        barrier_done = 1
                        packed = pp.tile([P, n_src, CT, 2], F)
                        for ski in range(n_src):
                            for cp in range(CT):
                                nc.vector.reduce_sum(
                                    packed[:, ski, cp, 0:1],
                                    stat_sum[:, ski, cp, 0:nchunks], axis=AX.X)
                                nc.vector.reduce_sum(
                                    packed[:, ski, cp, 1:2],
                                    stat_sq[:, ski, cp, 0:nchunks], axis=AX.X)
                        no_cc = os.environ.get("KERNEL_NO_CC", "0") == "1"
                        nc.sync.dma_start(
                            cc_in[:, 0:n_src * CT * 2],
                            packed[:].rearrange("p a b c -> p (a b c)"))
                        if no_cc:
                            nc.sync.dma_start(cc_out[:, 0:n_src * CT * 2],
                                              cc_in[:, 0:n_src * CT * 2])
                        else:
                            nc.gpsimd.collective_compute(
                                "AllReduce", mybir.AluOpType.add,
                                ins=[cc_in.opt()], outs=[cc_out.opt()],
                                replica_groups=[list(range(N_CORES))],
                            )
                        red = pp.tile([P, n_src, CT, 2], F)
                        nc.sync.dma_start(
                            red[:].rearrange("p a b c -> p (a b c)"),
                            cc_out[:, 0:n_src * CT * 2])
                        # coefficient computation (tiny [P, CT] tensors)
                        invB = (1.0 / B) if not no_cc else (1.0 / BL)
                        sc1 = pp.tile([P, n_src, CT, 8], F)  # scratch
                        for ski, key in enumerate(src_keys):
                            mS = sc1[:, ski, :, 0]    # mean
                            vS = sc1[:, ski, :, 1]    # var
                            uS = sc1[:, ski, :, 2]    # var+eps
                            ruS = sc1[:, ski, :, 3]   # 1/(var+eps)
                            r1S = sc1[:, ski, :, 4]   # rsqrt(var+eps)
                            t5 = sc1[:, ski, :, 5]
                            t6 = sc1[:, ski, :, 6]
                            t7 = sc1[:, ski, :, 7]
                            sm = red[:, ski, :, 0]
                            sq_ = red[:, ski, :, 1]
                            nc.vector.tensor_scalar_mul(mS, sm, invB)
                            nc.vector.tensor_scalar_mul(vS, sq_, invB)
                            nc.vector.tensor_mul(t5, mS, mS)
                            nc.vector.tensor_sub(vS, vS, t5)
                            nc.vector.tensor_scalar_add(uS, vS, EPS)
                            nc.vector.reciprocal(ruS, uS)
                            nc.scalar.activation(r1S, ruS, AF.Sqrt)
                        for pe_i, e in enumerate(pool_edges):
                            ski = src_keys.index(e["src_key"])
                            mS = sc1[:, ski, :, 0]
                            vS = sc1[:, ski, :, 1]
                            ruS = sc1[:, ski, :, 3]
                            r1S = sc1[:, ski, :, 4]
                            t5 = sc1[:, ski, :, 5]
                            t6 = sc1[:, ski, :, 6]
                            t7 = sc1[:, ski, :, 7]
                            scl = coef[:, pe_i, :, 0]
                            cst = coef[:, pe_i, :, 1]
                            if e["kind"] == "A":
                                # v2 = v/(v+eps); avg: v2 /= 81
                                nc.vector.tensor_mul(t5, vS, ruS)
                                if e["op"] == 0:
                                    nc.vector.tensor_scalar_mul(
                                        t5, t5, 1.0 / 81.0)
                                nc.vector.tensor_scalar_add(t5, t5, EPS)
                                nc.vector.reciprocal(t6, t5)
                                nc.scalar.activation(t7, t6, AF.Sqrt)
                                nc.vector.tensor_mul(scl, r1S, t7)
                                if e["op"] == 0:
                                    nc.vector.tensor_scalar_mul(
                                        scl, scl, 1.0 / 9.0)
                            else:
                                if e["op"] == 0:
                                    nc.vector.tensor_scalar_mul(
                                        t5, vS, 1.0 / 81.0)
                                    nc.vector.tensor_scalar_add(t5, t5, EPS)
                                    nc.vector.reciprocal(t6, t5)
                                    nc.scalar.activation(t7, t6, AF.Sqrt)
                                    nc.vector.tensor_scalar_mul(
                                        scl, t7, 1.0 / 9.0)
                                else:
                                    nc.vector.tensor_copy(scl, r1S)
                            if not e["w_one"]:
                                nc.vector.tensor_scalar_mul(
                                    scl, scl,
                                    scale_sb[:1, e["row"]:e["row"] + 1]
                                    .partition_broadcast(P))
                            nc.vector.tensor_mul(cst, mS, scl)
                            nc.vector.tensor_scalar_mul(cst, cst, -1.0)
                        for di, dnode in enumerate(pool_dests):
                            dps = [pe_i for pe_i, e in enumerate(pool_edges)
                                   if e["dst"] == dnode]
                            nc.vector.tensor_copy(ctot[:, di, :],
                                                  coef[:, dps[0], :, 1])
                            for pe_i in dps[1:]:
                                nc.vector.tensor_add(ctot[:, di, :],
                                                     ctot[:, di, :],
                                                     coef[:, pe_i, :, 1])

    nc.compile()
    return nc


# ---------------------------------------------------------------------------
# Host-side weight packing
# ---------------------------------------------------------------------------

def _pack_inputs(edges, inputs):
    arrs = {}
    bias_list = []
    for e in edges:
        op = e["op"]
        ei = e["row"]
        k = e["kind"]
        slot = e["slot"]
        if op in (2, 3, 4):
            a = op - 2
            W = np.asarray(inputs["dense_w_A" if k == "A" else "dense_w_B"]
                           )[slot, a]
            bias = np.asarray(inputs["dense_b_A" if k == "A" else "dense_b_B"]
                              )[slot, a]
            kt = XT if k == "A" else CT
            arrs[f"w{ei}"] = np.ascontiguousarray(
                W.T.reshape(kt, P, CH).astype(np.float32))
            bias_list.append((ei, bias))
        elif op in (5, 6, 7):
            a = op - 5
            gw = np.asarray(inputs["group_w_A" if k == "A" else "group_w_B"]
                            )[slot, a]  # [4, 128, cin_g]
            gb = np.asarray(inputs["group_b_A" if k == "A" else "group_b_B"]
                            )[slot, a]  # [512]
            ktg = 2 if k == "A" else 1
            wT = np.concatenate([gw[g].T for g in range(4)], axis=0)
            arrs[f"w{ei}"] = np.ascontiguousarray(
                wT.reshape(4 * ktg, P, P).astype(np.float32))
            bias_list.append((ei, gb))
        elif op in (0, 1) and k == "A":
            pw = np.asarray(inputs["pool_w_A"])[slot, op]  # [512, 1024]
            arrs[f"w{ei}"] = np.ascontiguousarray(
                pw.T.reshape(XT, P, CH).astype(np.float32))
    n_bias = max(len(bias_list), 1)
    biases = np.zeros((CT, P, n_bias), np.float32)
    col = 0
    for ei, b in bias_list:
        biases[:, :, col] = np.asarray(b, np.float32).reshape(CT, P)
        col += 1
    arrs["biases"] = biases
    return arrs


def kernel(**inputs):
    global LAST_RESULTS
    from concourse.bass_utils import run_bass_kernel_spmd

    index, w_sel = _routing(inputs["arch_params"], inputs["gumbel"])
    edges = _edge_list(index, w_sel)
    cfg = tuple((int(index[i]), bool(w_sel[i] == 1.0))
                for i in range(len(index)))

    if cfg not in _PROGRAM_CACHE:
        _PROGRAM_CACHE[cfg] = _build_program(cfg)
    nc = _PROGRAM_CACHE[cfg]

    arrs = _pack_inputs(edges, inputs)
    arrs["scales"] = np.asarray(w_sel, np.float32).reshape(1, -1)

    x = np.asarray(inputs["x"], np.float32)  # [B, C_IN]
    x_cores = x.reshape(N_CORES, BL, C_IN)

    in_maps = []
    for c in range(N_CORES):
        m = dict(arrs)
        m["x"] = np.ascontiguousarray(
            x_cores[c].T.reshape(XT, P, BL))
        in_maps.append(m)

    trace = os.environ.get("KERNEL_TRACE", "0") == "1"
    res = None
    for attempt in range(4):
        try:
            res = run_bass_kernel_spmd(nc, in_maps, list(range(N_CORES)),
                                       trace=trace)
            break
        except Exception:
            # the axon tunnel to the device pool is occasionally flaky
            # (transient "worker hung up" / INTERNAL); retry a few times
            if attempt == 3:
                raise
            import time as _time

            _time.sleep(5.0)
    LAST_RESULTS = res

    out = np.empty((B, CH), np.float32)
    for c in range(N_CORES):
        oc = res.results[c]["out"]  # [P, CT, BL]
        out[c * BL:(c + 1) * BL] = (
            oc.transpose(2, 1, 0).reshape(BL, CH))
    return out


# revision 16
# speedup vs baseline: 1.0874x; 1.0020x over previous
"""Trainium2 Bass kernel for nn_Classifier_1821066133734 (GDAS NAS cell).

Strategy
--------
* The gumbel-softmax routing is global (per-edge, not per-token): compute it
  on host in numpy, exactly mirroring the fp32 reference ops, and specialize
  the device program to the selected op per edge.  hardwts[e, index[e]] is
  (1 - p) + p in fp32 (== 1.0 for all realistic inputs); a generic scale path
  exists for the w != 1 case.
* Data-parallel over 8 NeuronCores: batch 16384 -> 2048 rows per core.
  All tensors live TRANSPOSED on device: channels on SBUF partitions, batch in
  the free dimension.  Host pre-transposes x and the selected weights, so the
  device program needs no transposes at all.
* Matmuls run as float32r (TF32-class, 1 cycle/row at N=512 -- measured 227ns
  per [128k x 128m x 512n] LDW+MM pair warm, vs 852ns for fp32).
* BatchNorm normalizes over the FULL batch.  BN only occurs inside pool ops.
  BN(BN(t)) collapses analytically: mean2 = 0, var2 = v1/(v1+eps), so every
  pool edge is a per-channel affine of a "source" tensor (t = relu(x) @ pwT
  for A-edges, the source state for B-edges) with coefficients derived from
  global per-channel sum/sumsq.  Those stats need one tiny AllReduce
  (n_pool_sources x 512 x 2 floats) across the 8 cores.
* Two-segment schedule: segment 1 computes nodes 1..Q (Q = max pool source),
  the pool source tensors and their local stats, spilling live states to DRAM;
  then the stats AllReduce; segment 2 computes nodes Q+1..6 including the
  pool affines, streaming the spilled states back per chunk.
"""

import os

import numpy as np

NODE_NUM = 6
C_IN = 1024
CH = 512
B = 16384
N_CORES = 8
BL = B // N_CORES  # 2048
N_OPS = 9
TAU = np.float32(10.0)
EPS = 1e-5
P = 128
CT = CH // P  # 4 channel partition-tiles
XT = C_IN // P  # 8

# Results of the traced run (filled when KERNEL_TRACE=1), for test.py.
LAST_RESULTS = None

_PROGRAM_CACHE = {}


# ---------------------------------------------------------------------------
# Host-side routing (numpy mirror of the jax reference)
# ---------------------------------------------------------------------------

def _routing(arch_params, gumbel):
    ap = np.asarray(arch_params, dtype=np.float32)
    gm = np.asarray(gumbel, dtype=np.float32)
    m = ap.max(axis=1, keepdims=True)
    s = ap - m
    lse = np.log(np.exp(s).sum(axis=1, keepdims=True), dtype=np.float32)
    logp = s - lse
    logits = (logp + gm) / TAU
    mm = logits.max(axis=1, keepdims=True)
    e = np.exp(logits - mm, dtype=np.float32)
    prob = e / e.sum(axis=1, keepdims=True)
    index = prob.argmax(axis=1)
    p = prob[np.arange(prob.shape[0]), index].astype(np.float32)
    w = (np.float32(1.0) - p) + p  # forward value of the straight-through wt
    return index, w


def _edge_list(index, w_sel):
    """Edges in reference iteration order with their selected op."""
    keys = sorted(
        "{}->{}".format(j, i + 1) for i in range(NODE_NUM) for j in range(i + 1)
    )
    e2i = {k: n for n, k in enumerate(keys)}
    edges = []
    a_ct = 0
    b_ct = 0
    for i in range(1, NODE_NUM + 1):
        for j in range(i):
            row = e2i["{}->{}".format(j, i)]
            kind = "A" if j == 0 else "B"
            rec = {
                "row": row,
                "src": j,
                "dst": i,
                "kind": kind,
                "slot": a_ct if kind == "A" else b_ct,
                "op": int(index[row]),
                "w": float(w_sel[row]),
            }
            if kind == "A":
                a_ct += 1
            else:
                b_ct += 1
            edges.append(rec)
    return edges


# ---------------------------------------------------------------------------
# Device program builder
# ---------------------------------------------------------------------------

def _plan(edges):
    """Segment plan: list of (nodes, barrier_after: bool)."""
    pool_edges = [e for e in edges if e["op"] in (0, 1)]
    if not pool_edges:
        return [list(range(1, NODE_NUM + 1))], []
    avail = [0 if e["kind"] == "A" else e["src"] for e in pool_edges]
    dests = [e["dst"] for e in pool_edges]
    q = max(avail)
    if q < min(dests):
        barriers = [q]
    else:
        # fallback: a barrier right before every pool dest (correct, slower)
        barriers = sorted({d - 1 for d in dests})
    segs = []
    prev = 0
    for bp in barriers:
        segs.append(list(range(prev + 1, bp + 1)))
        prev = bp
    segs.append(list(range(prev + 1, NODE_NUM + 1)))
    segs = [s for s in segs if s]
    return segs, barriers


def _build_program(cfg):
    """cfg: tuple of (op, w_is_one) per edge in reference order."""
    import concourse.mybir as mybir
    import concourse.tile as tile
    from concourse import bacc
    from concourse.bass import ts

    R = mybir.dt.float32r
    F = mybir.dt.float32
    AF = mybir.ActivationFunctionType
    ALU = mybir.AluOpType
    AX = mybir.AxisListType

    index = [c[0] for c in cfg]
    w_one = [c[1] for c in cfg]
    edges = _edge_list(index, [1.0] * len(index))
    # edges hold no numeric w; the actual scale value comes in through a small
    # "scales" input tensor so w != 1 doesn't force a recompile.
    for e in edges:
        e["w_one"] = bool(w_one[e["row"]])

    stage = int(os.environ.get("KERNEL_STAGE", "9"))
    feats = set(os.environ.get("KERNEL_FEAT", "").split(","))
    if stage <= 6:
        edges = [e for e in edges if e["dst"] <= stage]
        if stage <= 3:
            edges = [e for e in edges if e["op"] not in (0, 1)]
    segs, barriers = _plan(edges)
    if stage <= 6:
        segs = [[n for n in seg if n <= stage] for seg in segs]
        segs = [seg for seg in segs if seg]
    pool_edges = [e for e in edges if e["op"] in (0, 1)]
    # pool sources: ("t", edge_idx) for A pools, ("s", src_node) for B pools
    src_keys = []
    for e in pool_edges:
        key = ("t", e["row"]) if e["kind"] == "A" else ("s", e["src"])
        e["src_key"] = key
        if key not in src_keys:
            src_keys.append(key)
    n_src = len(src_keys)

    # which states must be spilled: used in a later segment than produced
    node_seg = {}
    for si, seg in enumerate(segs):
        for n in seg:
            node_seg[n] = si
    spill_states = set()
    if len(segs) > 1:
        for e in edges:
            if e["op"] == 8:
                continue
            if e["src"] >= 1 and node_seg[e["dst"]] > node_seg[e["src"]]:
                spill_states.add(e["src"])
        for key in src_keys:
            if key[0] == "s" and node_seg[key[1]] < len(segs) - 1:
                spill_states.add(key[1])

    nc = bacc.Bacc("TRN2", target_bir_lowering=False, debug=False,
                   num_devices=N_CORES)

    # ---- DRAM parameters -------------------------------------------------
    x_d = nc.declare_dram_parameter("x", [XT, P, BL], R, isOutput=False)
    w_d = {}
    n_bias = 0
    bias_col = {}
    for e in edges:
        op = e["op"]
        ei = e["row"]
        if op in (2, 3, 4):  # dense
            kt = XT if e["kind"] == "A" else CT
            w_d[ei] = nc.declare_dram_parameter(f"w{ei}", [kt, P, CH], R,
                                                isOutput=False)
            bias_col[ei] = n_bias
            n_bias += 1
        elif op in (5, 6, 7):  # grouped dense
            kt = 8 if e["kind"] == "A" else 4  # 4 groups x (2 or 1) ktiles
            w_d[ei] = nc.declare_dram_parameter(f"w{ei}", [kt, P, P], R,
                                                isOutput=False)
            bias_col[ei] = n_bias
            n_bias += 1
        elif op in (0, 1) and e["kind"] == "A":  # pool preprocess matmul
            w_d[ei] = nc.declare_dram_parameter(f"w{ei}", [XT, P, CH], R,
                                                isOutput=False)
    n_bias = max(n_bias, 1)
    bias_d = nc.declare_dram_parameter("biases", [CT, P, n_bias], F,
                                       isOutput=False)
    # per-edge scale (hardwts value); only read on the w != 1 path
    scale_d = nc.declare_dram_parameter("scales", [1, len(edges)], F,
                                        isOutput=False)
    out_d = nc.declare_dram_parameter("out", [P, CT, BL], F, isOutput=True)

    seg_of_edge = {}
    for e in edges:
        seg_of_edge[e["row"]] = node_seg[e["dst"]]

    with tile.TileContext(nc) as tc:
        with (
            tc.tile_pool(name="persist", bufs=1) as pp,
            tc.tile_pool(name="dram", bufs=1, space="DRAM") as dp,
        ):
            bias_sb = pp.tile([P, CT, n_bias], F)
            for ct in range(CT):
                nc.sync.dma_start(bias_sb[:, ct, :], bias_d[ct])
            scale_sb = None
            if not all(e["w_one"] for e in edges):
                scale_sb = pp.tile([P, len(edges)], F)
                nc.sync.dma_start(scale_sb[:1, :], scale_d[:])

            # spill DRAM tensors
            spill_d = {}
            for n in sorted(spill_states):
                spill_d[n] = dp.tile([P, CT, BL], R, name=f"spill_s{n}")
            t_spill_d = {}
            for key in src_keys:
                if key[0] == "t":
                    t_spill_d[key[1]] = dp.tile([P, CT, BL], F,
                                                name=f"spill_t{key[1]}")

            # stats columns: per source, per ct, [sum over chunks..., sq...]
            max_chunks = 4
            stat_sum = pp.tile([P, max(n_src, 1), CT, max_chunks], F)
            stat_sq = pp.tile([P, max(n_src, 1), CT, max_chunks], F)
            # collective buffers
            cc_in = dp.tile([P, max(n_src, 1) * CT * 2], F, name="cc_in")
            cc_out = dp.tile([P, max(n_src, 1) * CT * 2], F,
                             name="cc_out", addr_space="Shared")
            # BN affine coefficients: per pool edge (scale, const)
            coef = pp.tile([P, max(len(pool_edges), 1), CT, 2], F)
            # per-dest total const
            pool_dests = sorted({e["dst"] for e in pool_edges})
            ctot = pp.tile([P, max(len(pool_dests), 1), CT], F)

            states = {}  # node -> sbuf tile for current chunk (seg-local)

            def dense_mm(psum_t, w_sb, src_sb, kts, nb, nh, cp):
                # kt outer, batch-halves inner: both MMs of a half-pair share
                # one weight load (LDWEIGHTS amortization)
                for i, kt in enumerate(kts):
                    for h in range(nh):
                        nc.tensor.matmul(
                            psum_t[:, h * 512:(h + 1) * 512],
                            w_sb[:, kt, ts(cp, P)],
                            src_sb[:, kt, h * 512:h * 512 + 512],
                            start=(i == 0),
                            stop=(i == len(kts) - 1),
                        )

            def epilogue(e, psum_t, dest, ct, first, nb, act_idx,
                         tmp_pool, accum_out=None):
                """dest[:, ct, :nb] (+)= act(psum + bias) [* w]"""
                ei = e["row"]
                bias_ap = bias_sb[:, ct, bias_col[ei]:bias_col[ei] + 1]
                w1 = e["w_one"]
                dst_ap = dest[:, ct, 0:nb]
                if act_idx == 0:  # relu on DVE
                    if first and w1:
                        nc.vector.tensor_scalar(
                            dst_ap, psum_t, bias_ap, 0.0, ALU.add, ALU.max)
                    else:
                        tmp = tmp_pool.tile([P, nb], R, tag="tmp")
                        nc.vector.tensor_scalar(
                            tmp[:], psum_t, bias_ap, 0.0, ALU.add, ALU.max)
                        _accum(e, dst_ap, tmp[:], first)
                else:
                    func = AF.Sigmoid if act_idx == 1 else AF.Tanh
                    if first and w1:
                        nc.scalar.activation(dst_ap, psum_t, func,
                                             bias=bias_ap, scale=1.0,
                                             accum_out=accum_out)
                    else:
                        tmp = tmp_pool.tile([P, nb], R, tag="tmp")
                        nc.scalar.activation(tmp[:], psum_t, func,
                                             bias=bias_ap, scale=1.0)
                        _accum(e, dst_ap, tmp[:], first)

            def _accum(e, dst_ap, tmp_ap, first):
                if first:
                    if e["w_one"]:
                        nc.vector.tensor_copy(dst_ap, tmp_ap)
                    else:
                        nc.vector.tensor_scalar_mul(
                            dst_ap, tmp_ap,
                            scale_sb[:1, e["row"]:e["row"] + 1]
                            .partition_broadcast(P))
                else:
                    if e["w_one"]:
                        nc.vector.tensor_add(dst_ap, dst_ap, tmp_ap)
                    else:
                        nc.vector.scalar_tensor_tensor(
                            dst_ap, tmp_ap,
                            scale_sb[:1, e["row"]:e["row"] + 1]
                            .partition_broadcast(P),
                            dst_ap, ALU.mult, ALU.add)

            # ---------------- segments ----------------
            barrier_done = 0
            for si, seg in enumerate(segs):
                is_last_seg = si == len(segs) - 1
                nb = min(1024, BL)
                nchunks = BL // nb
                nh = nb // 512

                seg_edges = [e for e in edges
                             if node_seg[e["dst"]] == si and e["op"] != 8]
                # pool-source t edges whose stats barrier is at end of this seg
                t_edges = [e for e in pool_edges
                           if e["kind"] == "A" and si == 0]
                need_x = any(e["src"] == 0 and e["op"] != 8 for e in seg_edges) \
                    or bool(t_edges)

                with (
                    tc.tile_pool(name=f"w{si}", bufs=1) as wp,
                    tc.tile_pool(name=f"st{si}", bufs=1) as sp,
                    tc.tile_pool(name=f"tmp{si}", bufs=3) as tp,
                    tc.tile_pool(name=f"ps{si}", bufs=3,
                                 space="PSUM") as psp,
                ):
                    # -- weights: preload when the segment set is small,
                    #    stream per chunk through a shared 2-slot pool when big
                    seg_w_edges = []
                    seen_w = set()
                    for e in [x for x in seg_edges
                              if x["op"] not in (0, 1)] + t_edges:
                        ei = e["row"]
                        if ei in w_d and ei not in seen_w:
                            seen_w.add(ei)
                            seg_w_edges.append(ei)
                    w_kb = sum(w_d[ei].shape[0] * w_d[ei].shape[2] * 4 // 1024
                               for ei in seg_w_edges)
                    stream_w = w_kb > 60
                    w_sb = {}

                    def load_w(ei):
                        shp = w_d[ei].shape
                        t_w = wp.tile([P, shp[0], shp[2]], R,
                                      name=f"wsb{ei}",
                                      tag=("wstream" if stream_w
                                           else f"wsb{ei}"),
                                      bufs=(2 if stream_w else 1))
                        for kt in range(shp[0]):
                            nc.sync.dma_start(t_w[:, kt, :], w_d[ei][kt])
                        return t_w

                    if not stream_w:
                        for ei in seg_w_edges:
                            w_sb[ei] = load_w(ei)

                    for c in range(nchunks):
                        if stream_w:
                            w_sb = {}
                        bsl = slice(c * nb, (c + 1) * nb)
                        # lazy loads: emit each DMA at first use so the
                        # sync-queue order matches compute order (avoids the
                        # PE stalling on a FIFO of not-yet-needed transfers)
                        _x_tile = [None]

                        def get_x():
                            if _x_tile[0] is None:
                                xt_ = sp.tile([P, XT, nb], R, tag="x")
                                for kt in range(XT):
                                    nc.sync.dma_start(xt_[:, kt, :],
                                                      x_d[kt, :, bsl])
                                _x_tile[0] = xt_
                            return _x_tile[0]

                        _reloaded = set()

                        def get_state(n):
                            if node_seg[n] < si and n not in _reloaded:
                                _reloaded.add(n)
                                st = sp.tile([P, CT, nb], R, tag=f"s{n}")
                                nc.sync.dma_start(st[:],
                                                  spill_d[n][:, :, bsl])
                                states[n] = st
                            return states[n]

                        t_tiles = {}

                        def get_t(row):
                            if row not in t_tiles:
                                tt = sp.tile([P, CT, nb], F, tag=f"t{row}")
                                nc.sync.dma_start(
                                    tt[:], t_spill_d[row][:, :, bsl])
                                t_tiles[row] = tt
                            return t_tiles[row]

                        # -- compute nodes --
                        for node in seg:
                            n_edges = [e for e in edges if e["dst"] == node
                                       and e["op"] != 8]
                            reg = [e for e in n_edges if e["op"] not in (0, 1)]
                            pools = [e for e in n_edges if e["op"] in (0, 1)]
                            last_node = segs[-1][-1]
                            if node < last_node:
                                acc = sp.tile([P, CT, nb], R, tag=f"s{node}")
                            else:
                                acc = sp.tile([P, CT, nb], F, tag="acc_f")
                            states[node] = acc
                            first = [True] * CT
                            if "nopoolaff" in feats:
                                pools = []
                            v_tile = None
                            if pools:
                                # hoist: v = src*scale + Ctot depends only on
                                # the (re)loaded source + coefficients, so it
                                # can overlap this node's matmuls
                                e = pools[-1]
                                pe_i = pool_edges.index(e)
                                if e["kind"] == "A":
                                    src_t = (get_t(e["row"]) if si > 0
                                             else t_tiles[e["row"]])
                                else:
                                    src_t = get_state(e["src"])
                                di = pool_dests.index(node)
                                v_tile = sp.tile([P, CT, nb], F, tag="vaff")
                                for ct in range(CT):
                                    nc.vector.tensor_scalar(
                                        v_tile[:, ct, :], src_t[:, ct, :],
                                        coef[:, pe_i, ct, 0:1],
                                        ctot[:, di, ct:ct + 1],
                                        ALU.mult, ALU.add)
                            for e in reg:
                                op = e["op"]
                                src_sb = get_x() if e["src"] == 0 \
                                    else get_state(e["src"])
                                act_idx = op - 2 if op in (2, 3, 4) else op - 5
                                if op in (2, 3, 4):  # dense
                                    kts = list(range(
                                        XT if e["kind"] == "A" else CT))
                                    if e["row"] not in w_sb:
                                        w_sb[e["row"]] = load_w(e["row"])
                                    for cp in range(CT):
                                        ps = psp.tile([P, nb], F, tag="ps")
                                        dense_mm(ps, w_sb[e["row"]], src_sb,
                                                 kts, nb, nh, cp)
                                        epilogue(e, ps[:], acc, cp,
                                                 first[cp], nb, act_idx, tp)
                                        first[cp] = False
                                else:  # grouped dense
                                    ktg = 2 if e["kind"] == "A" else 1
                                    if e["row"] not in w_sb:
                                        w_sb[e["row"]] = load_w(e["row"])
                                    for g in range(4):
                                        ps = psp.tile([P, nb], F, tag="ps")
                                        for i in range(ktg):
                                            for h in range(nh):
                                                h0 = h * 512
                                                nc.tensor.matmul(
                                                    ps[:, h0:h0 + 512],
                                                    w_sb[e["row"]][:, g * ktg + i, :],
                                                    src_sb[:, g * ktg + i,
                                                           h0:h0 + 512],
                                                    start=(i == 0),
                                                    stop=(i == ktg - 1),
                                                )
                                        epilogue(e, ps[:], acc, g,
                                                 first[g], nb, act_idx, tp)
                                        first[g] = False
                            # pool affines (coefficients ready after barrier)
                            for pi, e in enumerate(pools[:-1]):
                                pe_i = pool_edges.index(e)
                                if e["kind"] == "A":
                                    src_t = (get_t(e["row"]) if si > 0
                                             else t_tiles[e["row"]])
                                else:
                                    src_t = get_state(e["src"])
                                for ct in range(CT):
                                    assert not first[ct]
                                    nc.vector.scalar_tensor_tensor(
                                        acc[:, ct, :], src_t[:, ct, :],
                                        coef[:, pe_i, ct, 0:1],
                                        acc[:, ct, :], ALU.mult, ALU.add)
                            if v_tile is not None:
                                for ct in range(CT):
                                    if first[ct]:
                                        nc.vector.tensor_copy(
                                            acc[:, ct, :], v_tile[:, ct, :])
                                    else:
                                        nc.vector.tensor_add(
                                            acc[:, ct, :], acc[:, ct, :],
                                            v_tile[:, ct, :])
                                for ct in range(CT):
                                    first[ct] = False
                            if node == last_node:
                                nc.gpsimd.dma_start(out_d[:, :, bsl], acc[:])

                        # -- pool source t tensors + stats (first segment) --
                        if "nostats" in feats:
                            t_edges = []
                        if si == 0 and t_edges:
                            # relu(x) in place (x is dead after this)
                            x_sb = get_x()
                            for kt in range(XT):
                                nc.vector.tensor_scalar_max(
                                    x_sb[:, kt, :], x_sb[:, kt, :], 0.0)
                        for e in t_edges:
                            tt = sp.tile([P, CT, nb], F, tag=f"t{e['row']}")
                            ski = src_keys.index(e["src_key"])
                            if e["row"] not in w_sb:
                                w_sb[e["row"]] = load_w(e["row"])
                            for cp in range(CT):
                                ps = psp.tile([P, nb], F, tag="ps")
                                dense_mm(ps, w_sb[e["row"]], x_sb,
                                         list(range(XT)), nb, nh, cp)
                                nc.scalar.activation(
                                    tt[:, cp, :], ps[:], AF.Copy,
                                    bias=0.0, scale=1.0,
                                    accum_out=(None if "noacc" in feats else
                                               stat_sum[:, ski, cp, c:c + 1]))
                                if "nottr" not in feats:
                                    sq = tp.tile([P, nb], F, tag="tmp")
                                    nc.vector.scalar_tensor_tensor(
                                        sq[:], tt[:, cp, :], 1.0, tt[:, cp, :],
                                        ALU.mult, ALU.mult,
                                        accum_out=stat_sq[:, ski, cp, c:c + 1])
                            nc.gpsimd.dma_start(
                                t_spill_d[e["row"]][:, :, bsl], tt[:])
                        # -- stats of B-pool source states in this segment --
                        if si < len(segs) - 1 and "nostats" not in feats:
                            for ski, key in enumerate(src_keys):
                                if key[0] != "s" or node_seg[key[1]] != si:
                                    continue
                                st = states[key[1]]
                                for cp in range(CT):
                                    nc.vector.reduce_sum(
                                        stat_sum[:, ski, cp, c:c + 1],
                                        st[:, cp, :].bitcast(F), axis=AX.X)
                                    if "nottr" not in feats:
                                        sq = tp.tile([P, nb], F, tag="tmp")
                                        nc.vector.scalar_tensor_tensor(
                                            sq[:], st[:, cp, :].bitcast(F), 1.0,
                                            st[:, cp, :].bitcast(F),
                                            ALU.mult, ALU.mult,
                                            accum_out=stat_sq[:, ski, cp, c:c + 1])
                        # -- spill states produced here and needed later --
                        # (SWDGE queue: keeps the sync queue free for the next
                        # segment's latency-critical reloads)
                        for n in seg:
                            if n in spill_states:
                                nc.gpsimd.dma_start(
                                    spill_d[n][:, :, bsl], states[n][:])

                    # ---- barrier: allreduce stats, compute coefficients ----
                    if si < len(segs) - 1 and n_src > 0 and barrier_done == 0 \
                            and "nostats" not in feats \
                            and "nobarrier" not in feats:
                        barrier_done = 1
                        packed = pp.tile([P, n_src, CT, 2], F)
                        for ski in range(n_src):
                            for cp in range(CT):
                                nc.vector.reduce_sum(
                                    packed[:, ski, cp, 0:1],
                                    stat_sum[:, ski, cp, 0:nchunks], axis=AX.X)
                                nc.vector.reduce_sum(
                                    packed[:, ski, cp, 1:2],
                                    stat_sq[:, ski, cp, 0:nchunks], axis=AX.X)
                        no_cc = os.environ.get("KERNEL_NO_CC", "0") == "1"
                        nc.sync.dma_start(
                            cc_in[:, 0:n_src * CT * 2],
                            packed[:].rearrange("p a b c -> p (a b c)"))
                        if no_cc:
                            nc.sync.dma_start(cc_out[:, 0:n_src * CT * 2],
                                              cc_in[:, 0:n_src * CT * 2])
                        else:
                            nc.gpsimd.collective_compute(
                                "AllReduce", mybir.AluOpType.add,
                                ins=[cc_in.opt()], outs=[cc_out.opt()],
                                replica_groups=[list(range(N_CORES))],
                            )
                        red = pp.tile([P, n_src, CT, 2], F)
                        nc.sync.dma_start(
                            red[:].rearrange("p a b c -> p (a b c)"),
                            cc_out[:, 0:n_src * CT * 2])
                        # coefficient computation (tiny [P, CT] tensors)
                        invB = (1.0 / B) if not no_cc else (1.0 / BL)
                        sc1 = pp.tile([P, n_src, CT, 8], F)  # scratch
                        for ski, key in enumerate(src_keys):
                            mS = sc1[:, ski, :, 0]    # mean
                            vS = sc1[:, ski, :, 1]    # var
                            uS = sc1[:, ski, :, 2]    # var+eps
                            ruS = sc1[:, ski, :, 3]   # 1/(var+eps)
                            r1S = sc1[:, ski, :, 4]   # rsqrt(var+eps)
                            t5 = sc1[:, ski, :, 5]
                            t6 = sc1[:, ski, :, 6]
                            t7 = sc1[:, ski, :, 7]
                            sm = red[:, ski, :, 0]
                            sq_ = red[:, ski, :, 1]
                            nc.vector.tensor_scalar_mul(mS, sm, invB)
                            nc.vector.tensor_scalar_mul(vS, sq_, invB)
                            nc.vector.tensor_mul(t5, mS, mS)
                            nc.vector.tensor_sub(vS, vS, t5)
                            nc.vector.tensor_scalar_add(uS, vS, EPS)
                            nc.vector.reciprocal(ruS, uS)
                            nc.scalar.activation(r1S, ruS, AF.Sqrt)
                        for pe_i, e in enumerate(pool_edges):
                            ski = src_keys.index(e["src_key"])
                            mS = sc1[:, ski, :, 0]
                            vS = sc1[:, ski, :, 1]
                            ruS = sc1[:, ski, :, 3]
                            r1S = sc1[:, ski, :, 4]
                            t5 = sc1[:, ski, :, 5]
                            t6 = sc1[:, ski, :, 6]
                            t7 = sc1[:, ski, :, 7]
                            scl = coef[:, pe_i, :, 0]
                            cst = coef[:, pe_i, :, 1]
                            if e["kind"] == "A":
                                # v2 = v/(v+eps); avg: v2 /= 81
                                nc.vector.tensor_mul(t5, vS, ruS)
                                if e["op"] == 0:
                                    nc.vector.tensor_scalar_mul(
                                        t5, t5, 1.0 / 81.0)
                                nc.vector.tensor_scalar_add(t5, t5, EPS)
                                nc.vector.reciprocal(t6, t5)
                                nc.scalar.activation(t7, t6, AF.Sqrt)
                                nc.vector.tensor_mul(scl, r1S, t7)
                                if e["op"] == 0:
                                    nc.vector.tensor_scalar_mul(
                                        scl, scl, 1.0 / 9.0)
                            else:
                                if e["op"] == 0:
                                    nc.vector.tensor_scalar_mul(
                                        t5, vS, 1.0 / 81.0)
                                    nc.vector.tensor_scalar_add(t5, t5, EPS)
                                    nc.vector.reciprocal(t6, t5)
                                    nc.scalar.activation(t7, t6, AF.Sqrt)
                                    nc.vector.tensor_scalar_mul(
                                        scl, t7, 1.0 / 9.0)
                                else:
                                    nc.vector.tensor_copy(scl, r1S)
                            if not e["w_one"]:
                                nc.vector.tensor_scalar_mul(
                                    scl, scl,
                                    scale_sb[:1, e["row"]:e["row"] + 1]
                                    .partition_broadcast(P))
                            nc.vector.tensor_mul(cst, mS, scl)
                            nc.vector.tensor_scalar_mul(cst, cst, -1.0)
                        for di, dnode in enumerate(pool_dests):
                            dps = [pe_i for pe_i, e in enumerate(pool_edges)
                                   if e["dst"] == dnode]
                            nc.vector.tensor_copy(ctot[:, di, :],
                                                  coef[:, dps[0], :, 1])
                            for pe_i in dps[1:]:
                                nc.vector.tensor_add(ctot[:, di, :],
                                                     ctot[:, di, :],
                                                     coef[:, pe_i, :, 1])

    nc.compile()
    return nc


# ---------------------------------------------------------------------------
# Host-side weight packing
# ---------------------------------------------------------------------------

def _pack_inputs(edges, inputs):
    arrs = {}
    bias_list = []
    for e in edges:
        op = e["op"]
        ei = e["row"]
        k = e["kind"]
        slot = e["slot"]
        if op in (2, 3, 4):
            a = op - 2
            W = np.asarray(inputs["dense_w_A" if k == "A" else "dense_w_B"]
                           )[slot, a]
            bias = np.asarray(inputs["dense_b_A" if k == "A" else "dense_b_B"]
                              )[slot, a]
            kt = XT if k == "A" else CT
            arrs[f"w{ei}"] = np.ascontiguousarray(
                W.T.reshape(kt, P, CH).astype(np.float32))
            bias_list.append((ei, bias))
        elif op in (5, 6, 7):
            a = op - 5
            gw = np.asarray(inputs["group_w_A" if k == "A" else "group_w_B"]
                            )[slot, a]  # [4, 128, cin_g]
            gb = np.asarray(inputs["group_b_A" if k == "A" else "group_b_B"]
                            )[slot, a]  # [512]
            ktg = 2 if k == "A" else 1
            wT = np.concatenate([gw[g].T for g in range(4)], axis=0)
            arrs[f"w{ei}"] = np.ascontiguousarray(
                wT.reshape(4 * ktg, P, P).astype(np.float32))
            bias_list.append((ei, gb))
        elif op in (0, 1) and k == "A":
            pw = np.asarray(inputs["pool_w_A"])[slot, op]  # [512, 1024]
            arrs[f"w{ei}"] = np.ascontiguousarray(
                pw.T.reshape(XT, P, CH).astype(np.float32))
    n_bias = max(len(bias_list), 1)
    biases = np.zeros((CT, P, n_bias), np.float32)
    col = 0
    for ei, b in bias_list:
        biases[:, :, col] = np.asarray(b, np.float32).reshape(CT, P)
        col += 1
    arrs["biases"] = biases
    return arrs


def kernel(**inputs):
    global LAST_RESULTS
    from concourse.bass_utils import run_bass_kernel_spmd

    index, w_sel = _routing(inputs["arch_params"], inputs["gumbel"])
    edges = _edge_list(index, w_sel)
    cfg = tuple((int(index[i]), bool(w_sel[i] == 1.0))
                for i in range(len(index)))

    if cfg not in _PROGRAM_CACHE:
        _PROGRAM_CACHE[cfg] = _build_program(cfg)
    nc = _PROGRAM_CACHE[cfg]

    arrs = _pack_inputs(edges, inputs)
    arrs["scales"] = np.asarray(w_sel, np.float32).reshape(1, -1)

    x = np.asarray(inputs["x"], np.float32)  # [B, C_IN]
    x_cores = x.reshape(N_CORES, BL, C_IN)

    in_maps = []
    for c in range(N_CORES):
        m = dict(arrs)
        m["x"] = np.ascontiguousarray(
            x_cores[c].T.reshape(XT, P, BL))
        in_maps.append(m)

    trace = os.environ.get("KERNEL_TRACE", "0") == "1"
    res = None
    for attempt in range(4):
        try:
            res = run_bass_kernel_spmd(nc, in_maps, list(range(N_CORES)),
                                       trace=trace)
            break
        except Exception:
            # the axon tunnel to the device pool is occasionally flaky
            # (transient "worker hung up" / INTERNAL); retry a few times
            if attempt == 3:
                raise
            import time as _time

            _time.sleep(5.0)
    LAST_RESULTS = res

    out = np.empty((B, CH), np.float32)
    for c in range(N_CORES):
        oc = res.results[c]["out"]  # [P, CT, BL]
        out[c * BL:(c + 1) * BL] = (
            oc.transpose(2, 1, 0).reshape(BL, CH))
    return out


# revision 17
# speedup vs baseline: 1.1356x; 1.0443x over previous
"""Trainium2 Bass kernel for nn_Classifier_1821066133734 (GDAS NAS cell).

Strategy
--------
* The gumbel-softmax routing is global (per-edge, not per-token): compute it
  on host in numpy, exactly mirroring the fp32 reference ops, and specialize
  the device program to the selected op per edge.  hardwts[e, index[e]] is
  (1 - p) + p in fp32 (== 1.0 for all realistic inputs); a generic scale path
  exists for the w != 1 case.
* Data-parallel over 8 NeuronCores: batch 16384 -> 2048 rows per core.
  All tensors live TRANSPOSED on device: channels on SBUF partitions, batch in
  the free dimension.  Host pre-transposes x and the selected weights, so the
  device program needs no transposes at all.
* Matmuls run as float32r (TF32-class, 1 cycle/row at N=512 -- measured 227ns
  per [128k x 128m x 512n] LDW+MM pair warm, vs 852ns for fp32).
* BatchNorm normalizes over the FULL batch.  BN only occurs inside pool ops.
  BN(BN(t)) collapses analytically: mean2 = 0, var2 = v1/(v1+eps), so every
  pool edge is a per-channel affine of a "source" tensor (t = relu(x) @ pwT
  for A-edges, the source state for B-edges) with coefficients derived from
  global per-channel sum/sumsq.  Those stats need one tiny AllReduce
  (n_pool_sources x 512 x 2 floats) across the 8 cores.
* Two-segment schedule: segment 1 computes nodes 1..Q (Q = max pool source),
  the pool source tensors and their local stats, spilling live states to DRAM;
  then the stats AllReduce; segment 2 computes nodes Q+1..6 including the
  pool affines, streaming the spilled states back per chunk.
"""

import os

import numpy as np

NODE_NUM = 6
C_IN = 1024
CH = 512
B = 16384
N_CORES = 8
BL = B // N_CORES  # 2048
N_OPS = 9
TAU = np.float32(10.0)
EPS = 1e-5
P = 128
CT = CH // P  # 4 channel partition-tiles
XT = C_IN // P  # 8

# Results of the traced run (filled when KERNEL_TRACE=1), for test.py.
LAST_RESULTS = None

_PROGRAM_CACHE = {}


# ---------------------------------------------------------------------------
# Host-side routing (numpy mirror of the jax reference)
# ---------------------------------------------------------------------------

def _routing(arch_params, gumbel):
    ap = np.asarray(arch_params, dtype=np.float32)
    gm = np.asarray(gumbel, dtype=np.float32)
    m = ap.max(axis=1, keepdims=True)
    s = ap - m
    lse = np.log(np.exp(s).sum(axis=1, keepdims=True), dtype=np.float32)
    logp = s - lse
    logits = (logp + gm) / TAU
    mm = logits.max(axis=1, keepdims=True)
    e = np.exp(logits - mm, dtype=np.float32)
    prob = e / e.sum(axis=1, keepdims=True)
    index = prob.argmax(axis=1)
    p = prob[np.arange(prob.shape[0]), index].astype(np.float32)
    w = (np.float32(1.0) - p) + p  # forward value of the straight-through wt
    return index, w


def _edge_list(index, w_sel):
    """Edges in reference iteration order with their selected op."""
    keys = sorted(
        "{}->{}".format(j, i + 1) for i in range(NODE_NUM) for j in range(i + 1)
    )
    e2i = {k: n for n, k in enumerate(keys)}
    edges = []
    a_ct = 0
    b_ct = 0
    for i in range(1, NODE_NUM + 1):
        for j in range(i):
            row = e2i["{}->{}".format(j, i)]
            kind = "A" if j == 0 else "B"
            rec = {
                "row": row,
                "src": j,
                "dst": i,
                "kind": kind,
                "slot": a_ct if kind == "A" else b_ct,
                "op": int(index[row]),
                "w": float(w_sel[row]),
            }
            if kind == "A":
                a_ct += 1
            else:
                b_ct += 1
            edges.append(rec)
    return edges


# ---------------------------------------------------------------------------
# Device program builder
# ---------------------------------------------------------------------------

def _plan(edges):
    """Segment plan: list of (nodes, barrier_after: bool)."""
    pool_edges = [e for e in edges if e["op"] in (0, 1)]
    if not pool_edges:
        return [list(range(1, NODE_NUM + 1))], []
    avail = [0 if e["kind"] == "A" else e["src"] for e in pool_edges]
    dests = [e["dst"] for e in pool_edges]
    q = max(avail)
    if q < min(dests):
        barriers = [q]
    else:
        # fallback: a barrier right before every pool dest (correct, slower)
        barriers = sorted({d - 1 for d in dests})
    segs = []
    prev = 0
    for bp in barriers:
        segs.append(list(range(prev + 1, bp + 1)))
        prev = bp
    segs.append(list(range(prev + 1, NODE_NUM + 1)))
    segs = [s for s in segs if s]
    return segs, barriers


def _build_program(cfg):
    """cfg: tuple of (op, w_is_one) per edge in reference order."""
    import concourse.mybir as mybir
    import concourse.tile as tile
    from concourse import bacc
    from concourse.bass import ts

    R = mybir.dt.float32r
    F = mybir.dt.float32
    AF = mybir.ActivationFunctionType
    ALU = mybir.AluOpType
    AX = mybir.AxisListType

    index = [c[0] for c in cfg]
    w_one = [c[1] for c in cfg]
    edges = _edge_list(index, [1.0] * len(index))
    # edges hold no numeric w; the actual scale value comes in through a small
    # "scales" input tensor so w != 1 doesn't force a recompile.
    for e in edges:
        e["w_one"] = bool(w_one[e["row"]])

    stage = int(os.environ.get("KERNEL_STAGE", "9"))
    feats = set(os.environ.get("KERNEL_FEAT", "").split(","))
    if stage <= 6:
        edges = [e for e in edges if e["dst"] <= stage]
        if stage <= 3:
            edges = [e for e in edges if e["op"] not in (0, 1)]
    segs, barriers = _plan(edges)
    if stage <= 6:
        segs = [[n for n in seg if n <= stage] for seg in segs]
        segs = [seg for seg in segs if seg]
    pool_edges = [e for e in edges if e["op"] in (0, 1)]
    # pool sources: ("t", edge_idx) for A pools, ("s", src_node) for B pools
    src_keys = []
    for e in pool_edges:
        key = ("t", e["row"]) if e["kind"] == "A" else ("s", e["src"])
        e["src_key"] = key
        if key not in src_keys:
            src_keys.append(key)
    n_src = len(src_keys)

    # which states must be spilled: used in a later segment than produced
    node_seg = {}
    for si, seg in enumerate(segs):
        for n in seg:
            node_seg[n] = si
    spill_states = set()
    if len(segs) > 1:
        for e in edges:
            if e["op"] == 8:
                continue
            if e["src"] >= 1 and node_seg[e["dst"]] > node_seg[e["src"]]:
                spill_states.add(e["src"])
        for key in src_keys:
            if key[0] == "s" and node_seg[key[1]] < len(segs) - 1:
                spill_states.add(key[1])

    nc = bacc.Bacc("TRN2", target_bir_lowering=False, debug=False,
                   num_devices=N_CORES)

    # ---- DRAM parameters -------------------------------------------------
    x_d = nc.declare_dram_parameter("x", [XT, P, BL], R, isOutput=False)
    w_d = {}
    n_bias = 0
    bias_col = {}
    for e in edges:
        op = e["op"]
        ei = e["row"]
        if op in (2, 3, 4):  # dense
            kt = XT if e["kind"] == "A" else CT
            w_d[ei] = nc.declare_dram_parameter(f"w{ei}", [kt, P, CH], R,
                                                isOutput=False)
            bias_col[ei] = n_bias
            n_bias += 1
        elif op in (5, 6, 7):  # grouped dense
            kt = 8 if e["kind"] == "A" else 4  # 4 groups x (2 or 1) ktiles
            w_d[ei] = nc.declare_dram_parameter(f"w{ei}", [kt, P, P], R,
                                                isOutput=False)
            bias_col[ei] = n_bias
            n_bias += 1
        elif op in (0, 1) and e["kind"] == "A":  # pool preprocess matmul
            w_d[ei] = nc.declare_dram_parameter(f"w{ei}", [XT, P, CH], R,
                                                isOutput=False)
    n_bias = max(n_bias, 1)
    bias_d = nc.declare_dram_parameter("biases", [CT, P, n_bias], F,
                                       isOutput=False)
    # per-edge scale (hardwts value); only read on the w != 1 path
    scale_d = nc.declare_dram_parameter("scales", [1, len(edges)], F,
                                        isOutput=False)
    out_d = nc.declare_dram_parameter("out", [P, CT, BL], F, isOutput=True)

    seg_of_edge = {}
    for e in edges:
        seg_of_edge[e["row"]] = node_seg[e["dst"]]

    with tile.TileContext(nc) as tc:
        with (
            tc.tile_pool(name="persist", bufs=1) as pp,
            tc.tile_pool(name="dram", bufs=1, space="DRAM") as dp,
        ):
            bias_sb = pp.tile([P, CT, n_bias], F)
            for ct in range(CT):
                nc.sync.dma_start(bias_sb[:, ct, :], bias_d[ct])
            scale_sb = None
            if not all(e["w_one"] for e in edges):
                scale_sb = pp.tile([P, len(edges)], F)
                nc.sync.dma_start(scale_sb[:1, :], scale_d[:])

            # spill DRAM tensors
            spill_d = {}
            for n in sorted(spill_states):
                spill_d[n] = dp.tile([P, CT, BL], R, name=f"spill_s{n}")
            t_spill_d = {}
            for key in src_keys:
                if key[0] == "t":
                    t_spill_d[key[1]] = dp.tile([P, CT, BL], F,
                                                name=f"spill_t{key[1]}")

            # stats columns: per source, per ct, [sum over chunks..., sq...]
            max_chunks = 4
            stat_sum = pp.tile([P, max(n_src, 1), CT, max_chunks], F)
            stat_sq = pp.tile([P, max(n_src, 1), CT, max_chunks], F)
            # collective buffers
            cc_in = dp.tile([P, max(n_src, 1) * CT * 2], F, name="cc_in")
            cc_out = dp.tile([P, max(n_src, 1) * CT * 2], F,
                             name="cc_out", addr_space="Shared")
            # BN affine coefficients: per pool edge (scale, const)
            coef = pp.tile([P, max(len(pool_edges), 1), CT, 2], F)
            # per-dest total const
            pool_dests = sorted({e["dst"] for e in pool_edges})
            ctot = pp.tile([P, max(len(pool_dests), 1), CT], F)

            states = {}  # node -> sbuf tile for current chunk (seg-local)

            def dense_mm(psum_t, w_sb, src_sb, kts, nb, nh, cp):
                # kt outer, batch-halves inner: both MMs of a half-pair share
                # one weight load (LDWEIGHTS amortization)
                for i, kt in enumerate(kts):
                    for h in range(nh):
                        nc.tensor.matmul(
                            psum_t[:, h * 512:(h + 1) * 512],
                            w_sb[:, kt, ts(cp, P)],
                            src_sb[:, kt, h * 512:h * 512 + 512],
                            start=(i == 0),
                            stop=(i == len(kts) - 1),
                        )

            def epilogue(e, psum_t, dest, ct, first, nb, act_idx,
                         tmp_pool, accum_out=None):
                """dest[:, ct, :nb] (+)= act(psum + bias) [* w]"""
                ei = e["row"]
                bias_ap = bias_sb[:, ct, bias_col[ei]:bias_col[ei] + 1]
                w1 = e["w_one"]
                dst_ap = dest[:, ct, 0:nb]
                if act_idx == 0:  # relu on DVE
                    if first and w1:
                        nc.vector.tensor_scalar(
                            dst_ap, psum_t, bias_ap, 0.0, ALU.add, ALU.max)
                    else:
                        tmp = tmp_pool.tile([P, nb], R, tag="tmp")
                        nc.vector.tensor_scalar(
                            tmp[:], psum_t, bias_ap, 0.0, ALU.add, ALU.max)
                        _accum(e, dst_ap, tmp[:], first)
                else:
                    func = AF.Sigmoid if act_idx == 1 else AF.Tanh
                    if first and w1:
                        nc.scalar.activation(dst_ap, psum_t, func,
                                             bias=bias_ap, scale=1.0,
                                             accum_out=accum_out)
                    else:
                        tmp = tmp_pool.tile([P, nb], R, tag="tmp")
                        nc.scalar.activation(tmp[:], psum_t, func,
                                             bias=bias_ap, scale=1.0)
                        _accum(e, dst_ap, tmp[:], first)

            def _accum(e, dst_ap, tmp_ap, first):
                if first:
                    if e["w_one"]:
                        nc.vector.tensor_copy(dst_ap, tmp_ap)
                    else:
                        nc.vector.tensor_scalar_mul(
                            dst_ap, tmp_ap,
                            scale_sb[:1, e["row"]:e["row"] + 1]
                            .partition_broadcast(P))
                else:
                    if e["w_one"]:
                        nc.vector.tensor_add(dst_ap, dst_ap, tmp_ap)
                    else:
                        nc.vector.scalar_tensor_tensor(
                            dst_ap, tmp_ap,
                            scale_sb[:1, e["row"]:e["row"] + 1]
                            .partition_broadcast(P),
                            dst_ap, ALU.mult, ALU.add)

            # ---------------- segments ----------------
            barrier_done = 0
            for si, seg in enumerate(segs):
                is_last_seg = si == len(segs) - 1
                nb = min(1024, BL)
                nchunks = BL // nb
                nh = nb // 512

                seg_edges = [e for e in edges
                             if node_seg[e["dst"]] == si and e["op"] != 8]
                # pool-source t edges whose stats barrier is at end of this seg
                t_edges = [e for e in pool_edges
                           if e["kind"] == "A" and si == 0]
                need_x = any(e["src"] == 0 and e["op"] != 8 for e in seg_edges) \
                    or bool(t_edges)

                with (
                    tc.tile_pool(name=f"w{si}", bufs=1) as wp,
                    tc.tile_pool(name=f"st{si}", bufs=1) as sp,
                    tc.tile_pool(name=f"tmp{si}", bufs=3) as tp,
                    tc.tile_pool(name=f"ps{si}", bufs=3,
                                 space="PSUM") as psp,
                ):
                    # -- weights: preload when the segment set is small,
                    #    stream per chunk through a shared 2-slot pool when big
                    seg_w_edges = []
                    seen_w = set()
                    for e in [x for x in seg_edges
                              if x["op"] not in (0, 1)] + t_edges:
                        ei = e["row"]
                        if ei in w_d and ei not in seen_w:
                            seen_w.add(ei)
                            seg_w_edges.append(ei)
                    w_kb = sum(w_d[ei].shape[0] * w_d[ei].shape[2] * 4 // 1024
                               for ei in seg_w_edges)
                    stream_w = w_kb > 60
                    w_sb = {}

                    def load_w(ei):
                        shp = w_d[ei].shape
                        t_w = wp.tile([P, shp[0], shp[2]], R,
                                      name=f"wsb{ei}",
                                      tag=("wstream" if stream_w
                                           else f"wsb{ei}"),
                                      bufs=(2 if stream_w else 1))
                        for kt in range(shp[0]):
                            nc.sync.dma_start(t_w[:, kt, :], w_d[ei][kt])
                        return t_w

                    # weights load lazily at first use in both modes; in
                    # preload mode the per-edge tags keep them resident across
                    # chunks, but the DMAs queue in compute order so the first
                    # matmul isn't stuck behind the whole weight set

                    for c in range(nchunks):
                        if stream_w:
                            w_sb = {}
                        bsl = slice(c * nb, (c + 1) * nb)
                        # lazy loads: emit each DMA at first use so the
                        # sync-queue order matches compute order (avoids the
                        # PE stalling on a FIFO of not-yet-needed transfers)
                        _x_tile = [None]

                        def get_x():
                            if _x_tile[0] is None:
                                xt_ = sp.tile([P, XT, nb], R, tag="x")
                                for kt in range(XT):
                                    nc.sync.dma_start(xt_[:, kt, :],
                                                      x_d[kt, :, bsl])
                                _x_tile[0] = xt_
                            return _x_tile[0]

                        _reloaded = set()

                        def get_state(n):
                            if node_seg[n] < si and n not in _reloaded:
                                _reloaded.add(n)
                                st = sp.tile([P, CT, nb], R, tag=f"s{n}")
                                nc.sync.dma_start(st[:],
                                                  spill_d[n][:, :, bsl])
                                states[n] = st
                            return states[n]

                        t_tiles = {}

                        def get_t(row):
                            if row not in t_tiles:
                                tt = sp.tile([P, CT, nb], F, tag=f"t{row}")
                                nc.sync.dma_start(
                                    tt[:], t_spill_d[row][:, :, bsl])
                                t_tiles[row] = tt
                            return t_tiles[row]

                        # -- compute nodes --
                        for node in seg:
                            n_edges = [e for e in edges if e["dst"] == node
                                       and e["op"] != 8]
                            reg = [e for e in n_edges if e["op"] not in (0, 1)]
                            pools = [e for e in n_edges if e["op"] in (0, 1)]
                            last_node = segs[-1][-1]
                            if node < last_node:
                                acc = sp.tile([P, CT, nb], R, tag=f"s{node}")
                            else:
                                acc = sp.tile([P, CT, nb], F, tag="acc_f")
                            states[node] = acc
                            first = [True] * CT
                            if "nopoolaff" in feats:
                                pools = []
                            v_tile = None
                            if pools:
                                # hoist: v = src*scale + Ctot depends only on
                                # the (re)loaded source + coefficients, so it
                                # can overlap this node's matmuls
                                e = pools[-1]
                                pe_i = pool_edges.index(e)
                                if e["kind"] == "A":
                                    src_t = (get_t(e["row"]) if si > 0
                                             else t_tiles[e["row"]])
                                else:
                                    src_t = get_state(e["src"])
                                di = pool_dests.index(node)
                                v_tile = sp.tile([P, CT, nb], F, tag="vaff")
                                for ct in range(CT):
                                    nc.vector.tensor_scalar(
                                        v_tile[:, ct, :], src_t[:, ct, :],
                                        coef[:, pe_i, ct, 0:1],
                                        ctot[:, di, ct:ct + 1],
                                        ALU.mult, ALU.add)
                            for e in reg:
                                op = e["op"]
                                src_sb = get_x() if e["src"] == 0 \
                                    else get_state(e["src"])
                                act_idx = op - 2 if op in (2, 3, 4) else op - 5
                                if op in (2, 3, 4):  # dense
                                    kts = list(range(
                                        XT if e["kind"] == "A" else CT))
                                    if e["row"] not in w_sb:
                                        w_sb[e["row"]] = load_w(e["row"])
                                    for cp in range(CT):
                                        ps = psp.tile([P, nb], F, tag="ps")
                                        dense_mm(ps, w_sb[e["row"]], src_sb,
                                                 kts, nb, nh, cp)
                                        epilogue(e, ps[:], acc, cp,
                                                 first[cp], nb, act_idx, tp)
                                        first[cp] = False
                                else:  # grouped dense
                                    ktg = 2 if e["kind"] == "A" else 1
                                    if e["row"] not in w_sb:
                                        w_sb[e["row"]] = load_w(e["row"])
                                    for g in range(4):
                                        ps = psp.tile([P, nb], F, tag="ps")
                                        for i in range(ktg):
                                            for h in range(nh):
                                                h0 = h * 512
                                                nc.tensor.matmul(
                                                    ps[:, h0:h0 + 512],
                                                    w_sb[e["row"]][:, g * ktg + i, :],
                                                    src_sb[:, g * ktg + i,
                                                           h0:h0 + 512],
                                                    start=(i == 0),
                                                    stop=(i == ktg - 1),
                                                )
                                        epilogue(e, ps[:], acc, g,
                                                 first[g], nb, act_idx, tp)
                                        first[g] = False
                            # pool affines (coefficients ready after barrier)
                            for pi, e in enumerate(pools[:-1]):
                                pe_i = pool_edges.index(e)
                                if e["kind"] == "A":
                                    src_t = (get_t(e["row"]) if si > 0
                                             else t_tiles[e["row"]])
                                else:
                                    src_t = get_state(e["src"])
                                for ct in range(CT):
                                    assert not first[ct]
                                    nc.vector.scalar_tensor_tensor(
                                        acc[:, ct, :], src_t[:, ct, :],
                                        coef[:, pe_i, ct, 0:1],
                                        acc[:, ct, :], ALU.mult, ALU.add)
                            if v_tile is not None:
                                for ct in range(CT):
                                    if first[ct]:
                                        nc.vector.tensor_copy(
                                            acc[:, ct, :], v_tile[:, ct, :])
                                    else:
                                        nc.vector.tensor_add(
                                            acc[:, ct, :], acc[:, ct, :],
                                            v_tile[:, ct, :])
                                for ct in range(CT):
                                    first[ct] = False
                            if node == last_node:
                                nc.gpsimd.dma_start(out_d[:, :, bsl], acc[:])

                        # -- pool source t tensors + stats (first segment) --
                        if "nostats" in feats:
                            t_edges = []
                        if si == 0 and t_edges:
                            # relu(x) in place (x is dead after this)
                            x_sb = get_x()
                            for kt in range(XT):
                                nc.vector.tensor_scalar_max(
                                    x_sb[:, kt, :], x_sb[:, kt, :], 0.0)
                        for e in t_edges:
                            tt = sp.tile([P, CT, nb], F, tag=f"t{e['row']}")
                            ski = src_keys.index(e["src_key"])
                            if e["row"] not in w_sb:
                                w_sb[e["row"]] = load_w(e["row"])
                            for cp in range(CT):
                                ps = psp.tile([P, nb], F, tag="ps")
                                dense_mm(ps, w_sb[e["row"]], x_sb,
                                         list(range(XT)), nb, nh, cp)
                                nc.scalar.activation(
                                    tt[:, cp, :], ps[:], AF.Copy,
                                    bias=0.0, scale=1.0,
                                    accum_out=(None if "noacc" in feats else
                                               stat_sum[:, ski, cp, c:c + 1]))
                                if "nottr" not in feats:
                                    sq = tp.tile([P, nb], F, tag="tmp")
                                    nc.vector.scalar_tensor_tensor(
                                        sq[:], tt[:, cp, :], 1.0, tt[:, cp, :],
                                        ALU.mult, ALU.mult,
                                        accum_out=stat_sq[:, ski, cp, c:c + 1])
                            nc.gpsimd.dma_start(
                                t_spill_d[e["row"]][:, :, bsl], tt[:])
                        # -- stats of B-pool source states in this segment --
                        if si < len(segs) - 1 and "nostats" not in feats:
                            for ski, key in enumerate(src_keys):
                                if key[0] != "s" or node_seg[key[1]] != si:
                                    continue
                                st = states[key[1]]
                                for cp in range(CT):
                                    nc.vector.reduce_sum(
                                        stat_sum[:, ski, cp, c:c + 1],
                                        st[:, cp, :].bitcast(F), axis=AX.X)
                                    if "nottr" not in feats:
                                        sq = tp.tile([P, nb], F, tag="tmp")
                                        nc.vector.scalar_tensor_tensor(
                                            sq[:], st[:, cp, :].bitcast(F), 1.0,
                                            st[:, cp, :].bitcast(F),
                                            ALU.mult, ALU.mult,
                                            accum_out=stat_sq[:, ski, cp, c:c + 1])
                        # -- spill states produced here and needed later --
                        # (SWDGE queue: keeps the sync queue free for the next
                        # segment's latency-critical reloads)
                        for n in seg:
                            if n in spill_states:
                                nc.gpsimd.dma_start(
                                    spill_d[n][:, :, bsl], states[n][:])

                    # ---- barrier: allreduce stats, compute coefficients ----
                    if si < len(segs) - 1 and n_src > 0 and barrier_done == 0 \
                            and "nostats" not in feats \
                            and "nobarrier" not in feats:
                        barrier_done = 1
                        packed = pp.tile([P, n_src, CT, 2], F)
                        for ski in range(n_src):
                            for cp in range(CT):
                                nc.vector.reduce_sum(
                                    packed[:, ski, cp, 0:1],
                                    stat_sum[:, ski, cp, 0:nchunks], axis=AX.X)
                                nc.vector.reduce_sum(
                                    packed[:, ski, cp, 1:2],
                                    stat_sq[:, ski, cp, 0:nchunks], axis=AX.X)
                        no_cc = os.environ.get("KERNEL_NO_CC", "0") == "1"
                        nc.sync.dma_start(
                            cc_in[:, 0:n_src * CT * 2],
                            packed[:].rearrange("p a b c -> p (a b c)"))
                        if no_cc:
                            nc.sync.dma_start(cc_out[:, 0:n_src * CT * 2],
                                              cc_in[:, 0:n_src * CT * 2])
                        else:
                            nc.gpsimd.collective_compute(
                                "AllReduce", mybir.AluOpType.add,
                                ins=[cc_in.opt()], outs=[cc_out.opt()],
                                replica_groups=[list(range(N_CORES))],
                            )
                        red = pp.tile([P, n_src, CT, 2], F)
                        nc.sync.dma_start(
                            red[:].rearrange("p a b c -> p (a b c)"),
                            cc_out[:, 0:n_src * CT * 2])
                        # coefficient computation (tiny [P, CT] tensors)
                        invB = (1.0 / B) if not no_cc else (1.0 / BL)
                        sc1 = pp.tile([P, n_src, CT, 8], F)  # scratch
                        for ski, key in enumerate(src_keys):
                            mS = sc1[:, ski, :, 0]    # mean
                            vS = sc1[:, ski, :, 1]    # var
                            uS = sc1[:, ski, :, 2]    # var+eps
                            ruS = sc1[:, ski, :, 3]   # 1/(var+eps)
                            r1S = sc1[:, ski, :, 4]   # rsqrt(var+eps)
                            t5 = sc1[:, ski, :, 5]
                            t6 = sc1[:, ski, :, 6]
                            t7 = sc1[:, ski, :, 7]
                            sm = red[:, ski, :, 0]
                            sq_ = red[:, ski, :, 1]
                            nc.vector.tensor_scalar_mul(mS, sm, invB)
                            nc.vector.tensor_scalar_mul(vS, sq_, invB)
                            nc.vector.tensor_mul(t5, mS, mS)
                            nc.vector.tensor_sub(vS, vS, t5)
                            nc.vector.tensor_scalar_add(uS, vS, EPS)
                            nc.vector.reciprocal(ruS, uS)
                            nc.scalar.activation(r1S, ruS, AF.Sqrt)
                        for pe_i, e in enumerate(pool_edges):
                            ski = src_keys.index(e["src_key"])
                            mS = sc1[:, ski, :, 0]
                            vS = sc1[:, ski, :, 1]
                            ruS = sc1[:, ski, :, 3]
                            r1S = sc1[:, ski, :, 4]
                            t5 = sc1[:, ski, :, 5]
                            t6 = sc1[:, ski, :, 6]
                            t7 = sc1[:, ski, :, 7]
                            scl = coef[:, pe_i, :, 0]
                            cst = coef[:, pe_i, :, 1]
                            if e["kind"] == "A":
                                # v2 = v/(v+eps); avg: v2 /= 81
                                nc.vector.tensor_mul(t5, vS, ruS)
                                if e["op"] == 0:
                                    nc.vector.tensor_scalar_mul(
                                        t5, t5, 1.0 / 81.0)
                                nc.vector.tensor_scalar_add(t5, t5, EPS)
                                nc.vector.reciprocal(t6, t5)
                                nc.scalar.activation(t7, t6, AF.Sqrt)
                                nc.vector.tensor_mul(scl, r1S, t7)
                                if e["op"] == 0:
                                    nc.vector.tensor_scalar_mul(
                                        scl, scl, 1.0 / 9.0)
                            else:
                                if e["op"] == 0:
                                    nc.vector.tensor_scalar_mul(
                                        t5, vS, 1.0 / 81.0)
                                    nc.vector.tensor_scalar_add(t5, t5, EPS)
                                    nc.vector.reciprocal(t6, t5)
                                    nc.scalar.activation(t7, t6, AF.Sqrt)
                                    nc.vector.tensor_scalar_mul(
                                        scl, t7, 1.0 / 9.0)
                                else:
                                    nc.vector.tensor_copy(scl, r1S)
                            if not e["w_one"]:
                                nc.vector.tensor_scalar_mul(
                                    scl, scl,
                                    scale_sb[:1, e["row"]:e["row"] + 1]
                                    .partition_broadcast(P))
                            nc.vector.tensor_mul(cst, mS, scl)
                            nc.vector.tensor_scalar_mul(cst, cst, -1.0)
                        for di, dnode in enumerate(pool_dests):
                            dps = [pe_i for pe_i, e in enumerate(pool_edges)
                                   if e["dst"] == dnode]
                            nc.vector.tensor_copy(ctot[:, di, :],
                                                  coef[:, dps[0], :, 1])
                            for pe_i in dps[1:]:
                                nc.vector.tensor_add(ctot[:, di, :],
                                                     ctot[:, di, :],
                                                     coef[:, pe_i, :, 1])

    nc.compile()
    return nc


# ---------------------------------------------------------------------------
# Host-side weight packing
# ---------------------------------------------------------------------------

def _pack_inputs(edges, inputs):
    arrs = {}
    bias_list = []
    for e in edges:
        op = e["op"]
        ei = e["row"]
        k = e["kind"]
        slot = e["slot"]
        if op in (2, 3, 4):
            a = op - 2
            W = np.asarray(inputs["dense_w_A" if k == "A" else "dense_w_B"]
                           )[slot, a]
            bias = np.asarray(inputs["dense_b_A" if k == "A" else "dense_b_B"]
                              )[slot, a]
            kt = XT if k == "A" else CT
            arrs[f"w{ei}"] = np.ascontiguousarray(
                W.T.reshape(kt, P, CH).astype(np.float32))
            bias_list.append((ei, bias))
        elif op in (5, 6, 7):
            a = op - 5
            gw = np.asarray(inputs["group_w_A" if k == "A" else "group_w_B"]
                            )[slot, a]  # [4, 128, cin_g]
            gb = np.asarray(inputs["group_b_A" if k == "A" else "group_b_B"]
                            )[slot, a]  # [512]
            ktg = 2 if k == "A" else 1
            wT = np.concatenate([gw[g].T for g in range(4)], axis=0)
            arrs[f"w{ei}"] = np.ascontiguousarray(
                wT.reshape(4 * ktg, P, P).astype(np.float32))
            bias_list.append((ei, gb))
        elif op in (0, 1) and k == "A":
            pw = np.asarray(inputs["pool_w_A"])[slot, op]  # [512, 1024]
            arrs[f"w{ei}"] = np.ascontiguousarray(
                pw.T.reshape(XT, P, CH).astype(np.float32))
    n_bias = max(len(bias_list), 1)
    biases = np.zeros((CT, P, n_bias), np.float32)
    col = 0
    for ei, b in bias_list:
        biases[:, :, col] = np.asarray(b, np.float32).reshape(CT, P)
        col += 1
    arrs["biases"] = biases
    return arrs


def kernel(**inputs):
    global LAST_RESULTS
    from concourse.bass_utils import run_bass_kernel_spmd

    index, w_sel = _routing(inputs["arch_params"], inputs["gumbel"])
    edges = _edge_list(index, w_sel)
    cfg = tuple((int(index[i]), bool(w_sel[i] == 1.0))
                for i in range(len(index)))

    if cfg not in _PROGRAM_CACHE:
        _PROGRAM_CACHE[cfg] = _build_program(cfg)
    nc = _PROGRAM_CACHE[cfg]

    arrs = _pack_inputs(edges, inputs)
    arrs["scales"] = np.asarray(w_sel, np.float32).reshape(1, -1)

    x = np.asarray(inputs["x"], np.float32)  # [B, C_IN]
    x_cores = x.reshape(N_CORES, BL, C_IN)

    in_maps = []
    for c in range(N_CORES):
        m = dict(arrs)
        m["x"] = np.ascontiguousarray(
            x_cores[c].T.reshape(XT, P, BL))
        in_maps.append(m)

    trace = os.environ.get("KERNEL_TRACE", "0") == "1"
    res = None
    for attempt in range(4):
        try:
            res = run_bass_kernel_spmd(nc, in_maps, list(range(N_CORES)),
                                       trace=trace)
            break
        except Exception:
            # the axon tunnel to the device pool is occasionally flaky
            # (transient "worker hung up" / INTERNAL); retry a few times
            if attempt == 3:
                raise
            import time as _time

            _time.sleep(5.0)
    LAST_RESULTS = res

    out = np.empty((B, CH), np.float32)
    for c in range(N_CORES):
        oc = res.results[c]["out"]  # [P, CT, BL]
        out[c * BL:(c + 1) * BL] = (
            oc.transpose(2, 1, 0).reshape(BL, CH))
    return out
